# revision 1
# baseline (speedup 1.0000x reference)
"""Trainium2 Bass kernel for HIVNet GCN message passing (8-core SPMD).

Strategy:
  - Pad N=10000 nodes to 10240 = 80 blocks x 128; core c owns 10 dst-blocks.
  - Per layer: hs = h*rsqrt(deg) (per-node scale), hws = hs @ W[l] computed on
    the owned shard, AllGather of bf16 hws into a DRAM table on every core.
  - Edge aggregation: edges (with self loops) sorted by dst; per dst-block a
    bulk dma_gather pulls the src rows (bf16, 512B each) into SBUF tiles
    [128 edges, 256]; one-hot "sel" matrices (host-built, bf16) reduce each
    128-edge tile onto the 128 dst rows via TensorE matmuls accumulated in
    PSUM.  t = nrm * segsum(hws[src]) applied via per-partition ACT scale.
  - BatchNorm: partial sums/sumsq per core -> 2KB AllReduce -> scale/shift
    broadcast via rank-1 TensorE matmul; relu + residual on DVE.
  - Readout: graph mean-pool via one-hot pool matrices (transposed layout so
    MLP runs with weights as lhsT), 257-row AllReduce, 3-layer MLP on core 0.
"""

import sys

sys.path.insert(0, "/opt/trn_rl_repo")

from contextlib import ExitStack

import numpy as np
import ml_dtypes

from concourse import bass, mybir, bacc, tile, library_config
from concourse.bass_utils import run_bass_kernel_spmd
from concourse.masks import make_identity

NCORE = 8
P = 128
H = 256
L = 4
NF = 9
G = 256
N = 10000
BPC = 10                # dst blocks per core
NPC = BPC * P           # 1280 nodes per core
NPAD = NCORE * NPC      # 10240
BN_EPS = 1e-5

f32 = mybir.dt.float32
bf16 = mybir.dt.bfloat16
i16 = mybir.dt.int16
bfnp = ml_dtypes.bfloat16

FT = mybir.ActivationFunctionType
OP = mybir.AluOpType

_compiled = {}


# --------------------------------------------------------------------------
# host-side structural preprocessing (sorting / padding / one-hot layout)
# --------------------------------------------------------------------------

def _preprocess(x, edge_index, batch_ids, emb, W, gamma, beta,
                mlp_W1, mlp_b1, mlp_W2, mlp_b2, mlp_W3, mlp_b3):
    src = np.asarray(edge_index[0], np.int64)
    dst = np.asarray(edge_index[1], np.int64)
    # self loops for every real node (weight nrm[d]^2 == nrm[d]*nrm[d] folds in)
    src_all = np.concatenate([src, np.arange(N, dtype=np.int64)])
    dst_all = np.concatenate([dst, np.arange(N, dtype=np.int64)])
    order = np.argsort(dst_all, kind="stable")
    s_sorted = src_all[order].astype(np.int64)
    d_sorted = dst_all[order]

    deg = np.bincount(dst_all, minlength=NPAD).astype(np.float64)  # incl self
    nblk = NCORE * BPC
    cnt_blk = np.bincount(d_sorted // P, minlength=nblk)
    T_blk = int(np.ceil(cnt_blk.max() / P))
    NI = T_blk * P
    NIB = NI // 16

    idx_slots = np.zeros((nblk, NI), np.int16)
    dloc = np.full((nblk, NI), -1, np.int32)
    starts = np.searchsorted(d_sorted, np.arange(nblk) * P)
    ends = np.searchsorted(d_sorted, (np.arange(nblk) + 1) * P)
    for g in range(nblk):
        c = ends[g] - starts[g]
        idx_slots[g, :c] = s_sorted[starts[g]:ends[g]]
        dloc[g, :c] = d_sorted[starts[g]:ends[g]] - g * P

    # one-hot sel: [blk, T_blk, 128 slots, 128 dst_local] bf16
    sel = (dloc.reshape(nblk, T_blk, P)[..., None]
           == np.arange(P, dtype=np.int32)).astype(bfnp)

    # graph pool one-hot [node, graph]
    bids = np.asarray(batch_ids, np.int64)
    psel_full = np.zeros((NPAD, G), np.float32)
    psel_full[np.arange(N), bids] = 1.0

    x_np = np.zeros((NPAD, NF), np.float32)
    x_np[:N] = np.asarray(x, np.float64)

    # shared parameter tensors (layout for device)
    Wf = np.asarray(W, np.float32)                       # [L,H,H]
    W_lhsT = Wf.reshape(L, 2, P, H).transpose(2, 0, 1, 3).reshape(P, L * 2 * H)
    gb = np.concatenate([np.asarray(gamma, np.float32).reshape(-1),
                         np.asarray(beta, np.float32).reshape(-1)])[None, :]
    embf = np.asarray(emb, np.float32)
    emb0 = np.ascontiguousarray(embf[:, 0, :])
    emb1 = np.ascontiguousarray(embf[:, 1, :])
    w1 = np.asarray(mlp_W1, np.float32).reshape(2, P, P).transpose(1, 0, 2).reshape(P, 2 * P)
    w2 = np.asarray(mlp_W2, np.float32)                  # [128,64]
    w3 = np.asarray(mlp_W3, np.float32)                  # [64,1]
    b1 = np.asarray(mlp_b1, np.float32).reshape(P, 1)
    b2 = np.asarray(mlp_b2, np.float32).reshape(64, 1)
    b3 = np.asarray(mlp_b3, np.float32).reshape(1, 1)

    in_maps = []
    for c in range(NCORE):
        lo, hi = c * NPC, (c + 1) * NPC
        gsl = slice(c * BPC, (c + 1) * BPC)

        selc = sel[gsl].reshape(BPC * T_blk, P, P)
        selc = np.ascontiguousarray(selc.transpose(1, 0, 2)).reshape(P, BPC * T_blk * P)

        idxc = idx_slots[gsl].reshape(BPC, NI // 16, 16)
        idxc = idxc.transpose(0, 2, 1)                    # [BPC, 16, NI/16]
        idxc = np.tile(idxc, (1, 8, 1))                   # replicate to 128 parts
        idxc = np.ascontiguousarray(idxc.transpose(1, 0, 2)).reshape(P, BPC * NIB)

        degc = deg[lo:hi].reshape(BPC, P).T               # [P, BPC]
        maskc = (degc > 0).astype(np.float32)
        degc = np.maximum(degc, 1.0).astype(np.float32)

        pselc = psel_full[lo:hi].reshape(BPC, P, G)
        pselc = np.ascontiguousarray(pselc.transpose(1, 0, 2)).reshape(P, BPC * G)

        xTc = np.ascontiguousarray(x_np[lo:hi].T)         # [NF, NPC]

        in_maps.append(dict(
            selw=selc.astype(bfnp), idx=idxc.astype(np.int16),
            xT=xTc, deg=degc, mask=maskc, psel=pselc,
            W=W_lhsT.astype(bfnp), gb=gb, emb0=emb0, emb1=emb1,
            w1=w1, w2=w2, w3=w3, b1=b1, b2=b2, b3=b3,
        ))
    return T_blk, in_maps


# --------------------------------------------------------------------------
# device program
# --------------------------------------------------------------------------

def _build(T_blk, variant='full'):
    NI = T_blk * P
    NIB = NI // 16
    nc = bacc.Bacc(None, target_bir_lowering=False)

    d_sel = nc.dram_tensor("selw", [P, BPC * T_blk * P], bf16, kind="ExternalInput")
    d_idx = nc.dram_tensor("idx", [P, BPC * NIB], i16, kind="ExternalInput")
    d_xT = nc.dram_tensor("xT", [NF, NPC], f32, kind="ExternalInput")
    d_deg = nc.dram_tensor("deg", [P, BPC], f32, kind="ExternalInput")
    d_mask = nc.dram_tensor("mask", [P, BPC], f32, kind="ExternalInput")
    d_psel = nc.dram_tensor("psel", [P, BPC * G], f32, kind="ExternalInput")
    d_W = nc.dram_tensor("W", [P, L * 2 * H], bf16, kind="ExternalInput")
    d_gb = nc.dram_tensor("gb", [1, 2 * L * H], f32, kind="ExternalInput")
    d_emb0 = nc.dram_tensor("emb0", [NF, H], f32, kind="ExternalInput")
    d_emb1 = nc.dram_tensor("emb1", [NF, H], f32, kind="ExternalInput")
    d_w1 = nc.dram_tensor("w1", [P, 2 * P], f32, kind="ExternalInput")
    d_w2 = nc.dram_tensor("w2", [P, 64], f32, kind="ExternalInput")
    d_w3 = nc.dram_tensor("w3", [64, 1], f32, kind="ExternalInput")
    d_b1 = nc.dram_tensor("b1", [P, 1], f32, kind="ExternalInput")
    d_b2 = nc.dram_tensor("b2", [64, 1], f32, kind="ExternalInput")
    d_b3 = nc.dram_tensor("b3", [1, 1], f32, kind="ExternalInput")
    d_out = nc.dram_tensor("out", [1, G], f32, kind="ExternalOutput")

    rg = [list(range(NCORE))]

    with tile.TileContext(nc) as tc, ExitStack() as ctx:
        pers = ctx.enter_context(tc.tile_pool(name="pers", bufs=1))
        psA = ctx.enter_context(tc.tile_pool(name="psA", bufs=2, space="PSUM"))
        psB = ctx.enter_context(tc.tile_pool(name="psB", bufs=2, space="PSUM"))
        psC = ctx.enter_context(tc.tile_pool(name="psC", bufs=1, space="PSUM"))
        gpool = ctx.enter_context(tc.tile_pool(name="gpool", bufs=2))
        work = ctx.enter_context(tc.tile_pool(name="work", bufs=2))
        stream = ctx.enter_context(tc.tile_pool(name="stream", bufs=2))
        dram = ctx.enter_context(tc.tile_pool(name="dram", bufs=2, space="DRAM"))

        # ---- persistent SBUF state -------------------------------------
        sel_sb = pers.tile([P, BPC * T_blk * P], bf16, tag="sel")
        idx_sb = pers.tile([P, BPC * NIB], i16, tag="idx")
        deg_sb = pers.tile([P, BPC], f32, tag="deg")
        mask_sb = pers.tile([P, BPC], f32, tag="mask")
        W_sb = pers.tile([P, L * 2 * H], bf16, tag="W")
        gb_sb = pers.tile([1, 2 * L * H], f32, tag="gb")
        emb0_sb = pers.tile([NF, H], f32, tag="emb0")
        emb1_sb = pers.tile([NF, H], f32, tag="emb1")
        w1_sb = pers.tile([P, 2 * P], f32, tag="w1")
        w2_sb = pers.tile([P, 64], f32, tag="w2")
        w3_sb = pers.tile([64, 1], f32, tag="w3")
        b1_sb = pers.tile([P, 1], f32, tag="b1")
        b2_sb = pers.tile([64, 1], f32, tag="b2")
        b3_sb = pers.tile([1, 1], f32, tag="b3")

        h_sb = pers.tile([P, BPC * H], f32, tag="h")
        hsT_sb = pers.tile([P, BPC * 2 * P], bf16, tag="hsT")
        hws_sb = pers.tile([P, BPC * H], bf16, tag="hws")
        t_all = pers.tile([P, BPC * H], f32, tag="t_all")
        nrm_sb = pers.tile([P, BPC], f32, tag="nrm")
        acc_s = pers.tile([P, H], f32, tag="acc_s")
        acc_q = pers.tile([P, H], f32, tag="acc_q")
        D_sb = pers.tile([NF, H], f32, tag="D")
        base_rep = pers.tile([P, H], f32, tag="base_rep")
        a_rep = pers.tile([P, H], f32, tag="a_rep")
        c_rep = pers.tile([P, H], f32, tag="c_rep")
        ident_bf = pers.tile([P, P], bf16, tag="ident")
        ones9 = pers.tile([NF, 1], f32, tag="ones9")
        ones1 = pers.tile([1, P], f32, tag="ones1")
        ones128 = pers.tile([P, 1], f32, tag="ones128")
        stv = pers.tile([1, 2 * H], f32, tag="stv")
        scal = pers.tile([1, 8 * H], f32, tag="scal")

        # ---- DRAM bounce buffers ---------------------------------------
        ag_in = dram.tile([NPC, H], bf16, tag="ag_in")
        ag_out = dram.tile([NPAD, H], bf16, tag="ag_out")
        ar_in = dram.tile([1, 2 * H], f32, tag="ar_in")
        ar_out = dram.tile([1, 2 * H], f32, tag="ar_out")
        pr_in = dram.tile([2 * P + 1, G], f32, tag="pr_in")
        pr_out = dram.tile([2 * P + 1, G], f32, tag="pr_out")

        # ---- input loads ------------------------------------------------
        for t, d in [(sel_sb, d_sel), (idx_sb, d_idx),
                     (deg_sb, d_deg), (mask_sb, d_mask),
                     (W_sb, d_W), (gb_sb, d_gb), (emb0_sb, d_emb0),
                     (emb1_sb, d_emb1), (w1_sb, d_w1), (w2_sb, d_w2),
                     (w3_sb, d_w3), (b1_sb, d_b1), (b2_sb, d_b2),
                     (b3_sb, d_b3)]:
            nc.sync.dma_start(out=t[:], in_=d[:])

        nc.gpsimd.load_library(library_config.mlp)
        make_identity(nc, ident_bf[:])
        nc.vector.memset(ones9[:], 1.0)
        nc.vector.memset(ones1[:], 1.0)
        nc.vector.memset(ones128[:], 1.0)

        # nrm = rsqrt(deg) * mask
        rdeg = work.tile([P, BPC], f32, tag="rdeg")
        nc.vector.reciprocal(out=rdeg[:], in_=deg_sb[:])
        nc.scalar.activation(out=rdeg[:], in_=rdeg[:], func=FT.Sqrt)
        nc.vector.tensor_tensor(out=nrm_sb[:], in0=rdeg[:], in1=mask_sb[:], op=OP.mult)

        # encoder prep: D = emb1 - emb0 ; base = ones9^T @ emb0, broadcast
        nc.vector.tensor_tensor(out=D_sb[:], in0=emb1_sb[:], in1=emb0_sb[:], op=OP.subtract)
        ps_b = psB.tile([1, H], f32, tag="vec")
        nc.tensor.matmul(out=ps_b[:], lhsT=ones9[:], rhs=emb0_sb[:], start=True, stop=True)
        bvec = scal[:, 0:H]
        nc.vector.tensor_copy(out=bvec, in_=ps_b[:])
        ps_br = psB.tile([P, H], f32, tag="vec")
        nc.tensor.matmul(out=ps_br[:], lhsT=ones1[:], rhs=bvec, start=True, stop=True)
        nc.vector.tensor_copy(out=base_rep[:], in_=ps_br[:])

        def hslice(nb):
            return h_sb[:, nb * H:(nb + 1) * H]

        def emit_hs_transpose(nb):
            """hs = h*nrm (bf16), transpose both 128-halves into hsT_sb."""
            hs_bf = work.tile([P, H], bf16, tag="hs_bf")
            nc.vector.tensor_scalar_mul(hs_bf[:], hslice(nb), nrm_sb[:, nb:nb + 1])
            for k in range(2):
                pst = psB.tile([P, P], bf16, tag="pst")
                nc.tensor.transpose(out=pst[:], in_=hs_bf[:, k * P:(k + 1) * P],
                                    identity=ident_bf[:])
                nc.vector.tensor_copy(out=hsT_sb[:, (nb * 2 + k) * P:(nb * 2 + k + 1) * P],
                                      in_=pst[:])

        # encoder: h0 = base + xT^T @ D  (per block)
        for nb in range(BPC):
            xT_t = stream.tile([NF, P], f32, tag="xT_t")
            nc.sync.dma_start(out=xT_t[:], in_=d_xT[:, nb * P:(nb + 1) * P])
            ps_h = psA.tile([P, H], f32, tag="mm")
            nc.tensor.matmul(out=ps_h[:], lhsT=xT_t[:],
                             rhs=D_sb[:], start=True, stop=True)
            nc.vector.tensor_tensor(out=hslice(nb), in0=ps_h[:], in1=base_rep[:], op=OP.add)
            emit_hs_transpose(nb)

        if variant == "enc":
            nc.sync.dma_start(out=d_out[:], in_=h_sb[0:1, 0:G])
        # ---- layers -----------------------------------------------------
        nlayers = 0 if variant == "enc" else (1 if variant in ("ag", "gat", "agg", "l1") else L)
        for l in range(nlayers):
            # GEMM hws = hs @ W[l]  (lhsT = hsT halves, rhs = W k-halves)
            for nb in range(BPC):
                ps_g = psA.tile([P, H], f32, tag="mm")
                for k in range(2):
                    nc.tensor.matmul(
                        out=ps_g[:],
                        lhsT=hsT_sb[:, (nb * 2 + k) * P:(nb * 2 + k + 1) * P],
                        rhs=W_sb[:, (l * 2 + k) * H:(l * 2 + k + 1) * H],
                        start=(k == 0), stop=(k == 1))
                nc.vector.tensor_copy(out=hws_sb[:, nb * H:(nb + 1) * H], in_=ps_g[:])
                nc.sync.dma_start(out=ag_in[nb * P:(nb + 1) * P, :],
                                  in_=hws_sb[:, nb * H:(nb + 1) * H])
            nc.gpsimd.collective_compute(
                "AllGather", OP.bypass, replica_groups=rg,
                ins=[ag_in[:]], outs=[ag_out[:]])
            if variant == "ag":
                sbtmp = work.tile([1, G], bf16, tag="dbg")
                nc.sync.dma_start(out=sbtmp[:], in_=ag_out[0:1, 0:G])
                sbtmp2 = work.tile([1, G], f32, tag="dbg2")
                nc.vector.tensor_copy(out=sbtmp2[:], in_=sbtmp[:])
                nc.sync.dma_start(out=d_out[:], in_=sbtmp2[:])
                break

            nc.vector.memset(acc_s[:], 0.0)
            nc.vector.memset(acc_q[:], 0.0)

            T0 = (T_blk + 1) // 2
            chunks = [(0, T0), (T0, T_blk)]
            for nb in range(BPC):
                gts = []
                for (j0, j1) in chunks:
                    gath = gpool.tile([P, T0 * H], bf16, tag="gath")
                    nc.gpsimd.dma_gather(
                        out_ap=gath[:, :(j1 - j0) * H].rearrange("p (t h) -> p t h", h=H),
                        in_ap=ag_out[:],
                        idxs_ap=idx_sb[:, nb * NIB + j0 * 8:nb * NIB + j1 * 8],
                        num_idxs=(j1 - j0) * P, num_idxs_reg=(j1 - j0) * P,
                        elem_size=H, single_packet=False)
                    gts.append(gath)
                if variant == "gat":
                    gtmp = work.tile([1, G], bf16, tag="dbg")
                    nc.vector.tensor_copy(out=gtmp[:], in_=gts[0][0:1, 0:G])
                    gtmp2 = work.tile([1, G], f32, tag="dbg2")
                    nc.vector.tensor_copy(out=gtmp2[:], in_=gtmp[:])
                    nc.sync.dma_start(out=d_out[:], in_=gtmp2[:])
                    break
                ps_t = psA.tile([P, H], f32, tag="mm")
                for j in range(T_blk):
                    ti = nb * T_blk + j
                    ci = 0 if j < T0 else 1
                    jj = j if j < T0 else j - T0
                    nc.tensor.matmul(
                        out=ps_t[:],
                        lhsT=sel_sb[:, ti * P:(ti + 1) * P],
                        rhs=gts[ci][:, jj * H:(jj + 1) * H],
                        start=(j == 0), stop=(j == T_blk - 1))
                tsl = t_all[:, nb * H:(nb + 1) * H]
                nc.scalar.activation(out=tsl, in_=ps_t[:], func=FT.Copy,
                                     scale=nrm_sb[:, nb:nb + 1])
                sq = work.tile([P, H], f32, tag="tmp")
                nc.vector.tensor_tensor(out=sq[:], in0=tsl, in1=tsl, op=OP.mult)
                nc.vector.tensor_tensor(out=acc_s[:], in0=acc_s[:], in1=tsl, op=OP.add)
                nc.vector.tensor_tensor(out=acc_q[:], in0=acc_q[:], in1=sq[:], op=OP.add)

            if variant == "gat":
                break
            if variant == "agg":
                nc.sync.dma_start(out=d_out[:], in_=t_all[0:1, 0:G])
                break
            # stats: cross-partition reduce + AllReduce
            ps_s = psB.tile([1, 2 * H], f32, tag="vec")
            nc.tensor.matmul(out=ps_s[:, 0:H], lhsT=ones128[:], rhs=acc_s[:],
                             start=True, stop=True)
            nc.tensor.matmul(out=ps_s[:, H:2 * H], lhsT=ones128[:], rhs=acc_q[:],
                             start=True, stop=True)
            st_sb = scal[:, 6 * H:8 * H]
            nc.vector.tensor_copy(out=st_sb, in_=ps_s[:])
            nc.sync.dma_start(out=ar_in[:], in_=st_sb)
            nc.gpsimd.collective_compute(
                "AllReduce", OP.add, replica_groups=rg,
                ins=[ar_in[:]], outs=[ar_out[:]])
            nc.sync.dma_start(out=stv[:], in_=ar_out[:])

            # a = gamma*istd ; c = beta - mu*a   (all [1,H] lanes)
            mu = scal[:, H:2 * H]
            var = scal[:, 2 * H:3 * H]
            av = scal[:, 3 * H:4 * H]
            cv = scal[:, 4 * H:5 * H]
            msq = scal[:, 5 * H:6 * H]
            nc.vector.tensor_scalar_mul(mu, stv[:, 0:H], 1.0 / N)
            nc.vector.tensor_scalar_mul(var, stv[:, H:2 * H], 1.0 / N)
            nc.vector.tensor_tensor(out=msq, in0=mu, in1=mu, op=OP.mult)
            nc.vector.tensor_tensor(out=var, in0=var, in1=msq, op=OP.subtract)
            nc.vector.tensor_scalar_add(var, var, BN_EPS)
            nc.vector.reciprocal(out=var, in_=var)
            nc.scalar.activation(out=var, in_=var, func=FT.Sqrt)  # istd
            nc.vector.tensor_tensor(out=av, in0=var,
                                    in1=gb_sb[:, l * H:(l + 1) * H], op=OP.mult)
            nc.vector.tensor_tensor(out=msq, in0=mu, in1=av, op=OP.mult)
            nc.vector.tensor_tensor(out=cv, in0=gb_sb[:, (L + l) * H:(L + l + 1) * H],
                                    in1=msq, op=OP.subtract)
            ps_a = psB.tile([P, H], f32, tag="vec")
            nc.tensor.matmul(out=ps_a[:], lhsT=ones1[:], rhs=av, start=True, stop=True)
            nc.vector.tensor_copy(out=a_rep[:], in_=ps_a[:])
            ps_c = psB.tile([P, H], f32, tag="vec")
            nc.tensor.matmul(out=ps_c[:], lhsT=ones1[:], rhs=cv, start=True, stop=True)
            nc.vector.tensor_copy(out=c_rep[:], in_=ps_c[:])

            # h = relu(t*a + c) + h ; prepare hsT for next layer
            for nb in range(BPC):
                tsl = t_all[:, nb * H:(nb + 1) * H]
                u = work.tile([P, H], f32, tag="tmp")
                nc.vector.tensor_tensor(out=u[:], in0=tsl, in1=a_rep[:], op=OP.mult)
                nc.vector.tensor_tensor(out=u[:], in0=u[:], in1=c_rep[:], op=OP.add)
                r = work.tile([P, H], f32, tag="tmp2")
                nc.scalar.activation(out=r[:], in_=u[:], func=FT.Relu)
                nc.vector.tensor_tensor(out=hslice(nb), in0=hslice(nb), in1=r[:], op=OP.add)
                if l < L - 1:
                    emit_hs_transpose(nb)

        if variant == "l1":
            nc.sync.dma_start(out=d_out[:], in_=h_sb[0:1, 0:G])
        skip_pool = variant in ("enc", "ag", "gat", "agg", "l1")
        # ---- pooling ----------------------------------------------------
        if not skip_pool:
            ps_p0 = psC.tile([P, G], f32, tag="p0")
            ps_p1 = psC.tile([P, G], f32, tag="p1")
            ps_pc = psB.tile([1, G], f32, tag="vec")
            for nb in range(BPC):
                psel_t = stream.tile([P, G], f32, tag="psel_t")
                nc.sync.dma_start(out=psel_t[:], in_=d_psel[:, nb * G:(nb + 1) * G])
                pssl = psel_t[:]
                nc.tensor.matmul(out=ps_p0[:], lhsT=h_sb[:, nb * H:nb * H + P],
                                 rhs=pssl, start=(nb == 0), stop=(nb == BPC - 1))
                nc.tensor.matmul(out=ps_p1[:], lhsT=h_sb[:, nb * H + P:(nb + 1) * H],
                                 rhs=pssl, start=(nb == 0), stop=(nb == BPC - 1))
                nc.tensor.matmul(out=ps_pc[:], lhsT=ones128[:],
                                 rhs=pssl, start=(nb == 0), stop=(nb == BPC - 1))
            g0 = work.tile([P, G], f32, tag="g0")
            g1 = work.tile([P, G], f32, tag="g1")
            cnt = scal[:, 0:G]
            nc.vector.tensor_copy(out=g0[:], in_=ps_p0[:])
            nc.vector.tensor_copy(out=g1[:], in_=ps_p1[:])
            nc.vector.tensor_copy(out=cnt, in_=ps_pc[:])
            nc.sync.dma_start(out=pr_in[0:P, :], in_=g0[:])
            nc.sync.dma_start(out=pr_in[P:2 * P, :], in_=g1[:])
            nc.sync.dma_start(out=pr_in[2 * P:2 * P + 1, :], in_=cnt)
            nc.gpsimd.collective_compute(
                "AllReduce", OP.add, replica_groups=rg,
                ins=[pr_in[:]], outs=[pr_out[:]])
            nc.sync.dma_start(out=g0[:], in_=pr_out[0:P, :])
            nc.sync.dma_start(out=g1[:], in_=pr_out[P:2 * P, :])
            nc.sync.dma_start(out=cnt, in_=pr_out[2 * P:2 * P + 1, :])
            nc.vector.tensor_scalar_max(cnt, cnt, 1.0)
            nc.vector.reciprocal(out=cnt, in_=cnt)
            ps_r = psB.tile([P, G], f32, tag="vec")
            nc.tensor.matmul(out=ps_r[:], lhsT=ones1[:], rhs=cnt, start=True, stop=True)
            rc_rep = work.tile([P, G], f32, tag="rc_rep")
            nc.vector.tensor_copy(out=rc_rep[:], in_=ps_r[:])
            nc.vector.tensor_tensor(out=g0[:], in0=g0[:], in1=rc_rep[:], op=OP.mult)
            nc.vector.tensor_tensor(out=g1[:], in0=g1[:], in1=rc_rep[:], op=OP.mult)

            # MLP head (transposed: weights are lhsT, graphs along free dim)
            ps1 = psB.tile([P, G], f32, tag="vec")
            nc.tensor.matmul(out=ps1[:], lhsT=w1_sb[:, 0:P], rhs=g0[:], start=True, stop=False)
            nc.tensor.matmul(out=ps1[:], lhsT=w1_sb[:, P:2 * P], rhs=g1[:], start=False, stop=True)
            y1 = work.tile([P, G], f32, tag="y1")
            nc.scalar.activation(out=y1[:], in_=ps1[:], func=FT.Relu, bias=b1_sb[:, 0:1])
            ps2 = psB.tile([64, G], f32, tag="vec")
            nc.tensor.matmul(out=ps2[:], lhsT=w2_sb[:], rhs=y1[:], start=True, stop=True)
            y2 = work.tile([64, G], f32, tag="y2")
            nc.scalar.activation(out=y2[:], in_=ps2[:], func=FT.Relu, bias=b2_sb[:, 0:1])
            ps3 = psB.tile([1, G], f32, tag="vec")
            nc.tensor.matmul(out=ps3[:], lhsT=w3_sb[:], rhs=y2[:], start=True, stop=True)
            y3 = work.tile([1, G], f32, tag="y3")
            nc.vector.tensor_scalar_add(y3[:], ps3[:], b3_sb[0:1, 0:1])
            nc.sync.dma_start(out=d_out[:], in_=y3[:])

    nc.compile()
    return nc


# --------------------------------------------------------------------------
# entry point
# --------------------------------------------------------------------------

def kernel(x, edge_index, batch_ids, emb, W, b, gamma, beta,
           mlp_W1, mlp_b1, mlp_W2, mlp_b2, mlp_W3, mlp_b3,
           _trace=False, _trace_kwargs=None):
    # NB: reference BN subtracts the per-channel mean, so the additive bias b
    # cancels exactly and is not needed by the device program.
    T_blk, in_maps = _preprocess(x, edge_index, batch_ids, emb, W, gamma, beta,
                                 mlp_W1, mlp_b1, mlp_W2, mlp_b2, mlp_W3, mlp_b3)
    import os
    variant = os.environ.get("KVARIANT", "full")
    key = (T_blk, variant)
    if key not in _compiled:
        _compiled[key] = _build(T_blk, variant)
    nc = _compiled[key]
    kw = {}
    if _trace:
        kw = dict(trace=True, **(_trace_kwargs or {}))
    res = run_bass_kernel_spmd(nc, in_maps, core_ids=list(range(NCORE)), **kw)
    out = np.asarray(res.results[0]["out"], np.float32).reshape(G, 1)
    kernel._last_results = res
    return out



# revision 14
# speedup vs baseline: 1.6028x; 1.6028x over previous
"""Trainium2 Bass kernel for HIVNet GCN message passing (8-core SPMD).

V2 design (replaces dma_gather-based V1 whose GpSimd descriptor generation
was the bottleneck at ~18us per gather call, 1.45ms total):

  - h kept TRANSPOSED on-chip: hT [128 (h-half), 2 x 1280 nodes] f32.
  - Per layer: GEMM hws = (h*nrm) @ W[l] via lhsT = hsT blocks (no explicit
    transposes needed), result written bf16 to a DRAM table; AllGather with
    SHARED output (each core writes only its 655KB slice).
  - Edge aggregation as block-dense SpMM on TensorE: t^T[h, dst] =
    sum_s table[s, h] * A[s, dst], where A is the host-built [10240 x 1280]
    dst-shard adjacency-count matrix (self loops included), streamed from
    DRAM as fp8e4 (counts are small integers => exact). 320 matmuls/layer,
    ~85us PE time, zero GpSimd work.
  - BN stats via DVE tensor_tensor_reduce (fused nrm_dst scaling + row sums),
    2KB AllReduce, per-partition scale/bias applied with one ACT op per half.
  - Readout: transpose final h blocks back to node-major, pool one-hot
    matmuls, 257-row AllReduce, 3-layer MLP.
"""

import sys

sys.path.insert(0, "/opt/trn_rl_repo")

from contextlib import ExitStack

import numpy as np
import ml_dtypes

from concourse import bass, mybir, bacc, tile
from concourse.bass_utils import run_bass_kernel_spmd
from concourse.masks import make_identity

NCORE = 8
P = 128
H = 256
L = 4
NF = 9
G = 256
N = 10000
BPC = 10                # dst blocks per core
NPC = BPC * P           # 1280 nodes per core
NPAD = NCORE * NPC      # 10240
SB = NPAD // P          # 80 src blocks
PASSES = [(0, 512), (512, 1024), (1024, 1280)]  # dst column passes (<=512)
BN_EPS = 1e-5

f32 = mybir.dt.float32
bf16 = mybir.dt.bfloat16
fp8 = mybir.dt.float8e4
bfnp = ml_dtypes.bfloat16
fp8np = ml_dtypes.float8_e4m3fn

FT = mybir.ActivationFunctionType
OP = mybir.AluOpType

_compiled = {}


# --------------------------------------------------------------------------
# host-side structural preprocessing
# --------------------------------------------------------------------------

def _preprocess(x, edge_index, batch_ids, emb, W, gamma, beta,
                mlp_W1, mlp_b1, mlp_W2, mlp_b2, mlp_W3, mlp_b3):
    src = np.asarray(edge_index[0], np.int64)
    dst = np.asarray(edge_index[1], np.int64)

    deg = np.zeros(NPAD, np.float64)
    np.add.at(deg, dst, 1.0)
    deg[:N] += 1.0  # self loops
    nrm = np.zeros(NPAD, np.float32)
    nrm[:N] = 1.0 / np.sqrt(deg[:N])

    # per-core adjacency count matrices A_c [NPAD, NPC], fp8-exact ints
    order = np.argsort(dst, kind="stable")
    s_sorted = src[order]
    d_sorted = dst[order]
    bounds = np.searchsorted(d_sorted, np.arange(NCORE + 1) * NPC)

    x_np = np.zeros((NPAD, NF), np.float32)
    x_np[:N] = np.asarray(x, np.float64)

    bids = np.asarray(batch_ids, np.int64)
    psel_full = np.zeros((NPAD, G), np.float32)
    psel_full[np.arange(N), bids] = 1.0

    embf = np.asarray(emb, np.float32)
    emb0 = np.ascontiguousarray(embf[:, 0, :])                # [NF, H]
    D = np.ascontiguousarray(embf[:, 1, :] - embf[:, 0, :])   # [NF, H]
    base = emb0.sum(axis=0)                                   # [H]

    Wf = np.asarray(W, np.float32)                            # [L,H,H]
    W_r = Wf.reshape(L, 2, P, H).transpose(2, 0, 1, 3).reshape(P, L * 2 * H)

    gT = np.asarray(gamma, np.float32).reshape(L * 2, P).T    # [P, 2L]
    bT = np.asarray(beta, np.float32).reshape(L * 2, P).T     # [P, 2L]
    baseT = base.reshape(2, P).T                              # [P, 2]

    w1 = np.asarray(mlp_W1, np.float32).reshape(2, P, P).transpose(1, 0, 2).reshape(P, 2 * P)
    w2 = np.asarray(mlp_W2, np.float32)                       # [128,64]
    w3 = np.asarray(mlp_W3, np.float32)                       # [64,1]
    b1 = np.asarray(mlp_b1, np.float32).reshape(P, 1)
    b2 = np.asarray(mlp_b2, np.float32).reshape(64, 1)
    b3 = np.asarray(mlp_b3, np.float32).reshape(1, 1)

    in_maps = []
    for c in range(NCORE):
        lo, hi = c * NPC, (c + 1) * NPC

        es, ee = bounds[c], bounds[c + 1]
        a_idx = s_sorted[es:ee] * np.int64(NPC) + (d_sorted[es:ee] - lo)
        A = np.bincount(a_idx, minlength=NPAD * NPC).astype(np.float32)
        A = A.reshape(NPAD, NPC)
        own = np.arange(lo, min(hi, N), dtype=np.int64)
        A[own, own - lo] += 1.0  # self loops
        # src-block order: all cores' first half-shards, then second halves,
        # so SpMM on the first 40 slab slots only needs the first AllGather.
        perm = [k * BPC + h * (BPC // 2) + bb
                for h in range(2) for k in range(NCORE) for bb in range(BPC // 2)]
        A_blk = np.ascontiguousarray(
            A.reshape(SB, P, NPC)[perm].transpose(1, 0, 2)).reshape(P, SB * NPC)

        nrmT = np.ascontiguousarray(nrm[lo:hi]).reshape(1, NPC)
        xTc = np.ascontiguousarray(x_np[lo:hi].T)             # [NF, NPC]

        pselc = psel_full[lo:hi].reshape(BPC, P, G)
        pselc = np.ascontiguousarray(pselc.transpose(1, 0, 2)).reshape(P, BPC * G)

        in_maps.append(dict(
            A=A_blk.astype(fp8np), xT=xTc, nrmT=nrmT,
            D=D, baseT=baseT, W=W_r.astype(bfnp), gT=gT, bT=bT,
            psel=pselc.astype(bfnp),
            w1=w1, w2=w2, w3=w3, b1=b1, b2=b2, b3=b3,
        ))
    return in_maps


# --------------------------------------------------------------------------
# device program
# --------------------------------------------------------------------------

def _build(variant="full"):
    nc = bacc.Bacc(None, target_bir_lowering=False)

    d_A = nc.dram_tensor("A", [P, SB * NPC], fp8, kind="ExternalInput")
    d_xT = nc.dram_tensor("xT", [NF, NPC], f32, kind="ExternalInput")
    d_nrmT = nc.dram_tensor("nrmT", [1, NPC], f32, kind="ExternalInput")
    d_D = nc.dram_tensor("D", [NF, H], f32, kind="ExternalInput")
    d_baseT = nc.dram_tensor("baseT", [P, 2], f32, kind="ExternalInput")
    d_W = nc.dram_tensor("W", [P, L * 2 * H], bf16, kind="ExternalInput")
    d_gT = nc.dram_tensor("gT", [P, 2 * L], f32, kind="ExternalInput")
    d_bT = nc.dram_tensor("bT", [P, 2 * L], f32, kind="ExternalInput")
    d_psel = nc.dram_tensor("psel", [P, BPC * G], bf16, kind="ExternalInput")
    d_w1 = nc.dram_tensor("w1", [P, 2 * P], f32, kind="ExternalInput")
    d_w2 = nc.dram_tensor("w2", [P, 64], f32, kind="ExternalInput")
    d_w3 = nc.dram_tensor("w3", [64, 1], f32, kind="ExternalInput")
    d_b1 = nc.dram_tensor("b1", [P, 1], f32, kind="ExternalInput")
    d_b2 = nc.dram_tensor("b2", [64, 1], f32, kind="ExternalInput")
    d_b3 = nc.dram_tensor("b3", [1, 1], f32, kind="ExternalInput")
    d_out = nc.dram_tensor("out", [1, G], f32, kind="ExternalOutput")

    rg = [list(range(NCORE))]

    with tile.TileContext(nc) as tc, ExitStack() as ctx:
        pers = ctx.enter_context(tc.tile_pool(name="pers", bufs=1))
        psT = ctx.enter_context(tc.tile_pool(name="psT", bufs=1, space="PSUM"))
        psG = ctx.enter_context(tc.tile_pool(name="psG", bufs=2, space="PSUM"))
        psV = ctx.enter_context(tc.tile_pool(name="psV", bufs=2, space="PSUM"))
        stream = ctx.enter_context(tc.tile_pool(name="stream", bufs=3))
        work = ctx.enter_context(tc.tile_pool(name="work", bufs=2))
        dram = ctx.enter_context(tc.tile_pool(name="dram", bufs=1, space="DRAM"))

        # ---- persistent SBUF state -------------------------------------
        hT = pers.tile([P, 2 * NPC], f32, tag="hT")
        hsT = pers.tile([P, 2 * NPC], bf16, tag="hsT")
        tT = pers.tile([P, 2 * NPC], f32, tag="tT")
        nrm_rep = pers.tile([P, NPC], f32, tag="nrm_rep")
        tbl = pers.tile([P, SB * H], bf16, tag="tbl")
        xT_sb = pers.tile([NF, NPC], f32, tag="xT")
        D_sb = pers.tile([NF, H], f32, tag="D")
        baseT_sb = pers.tile([P, 2], f32, tag="baseT")
        nrmT_sb = pers.tile([1, NPC], f32, tag="nrmT")
        W_sb = pers.tile([P, L * 2 * H], bf16, tag="W")
        gT_sb = pers.tile([P, 2 * L], f32, tag="gT")
        bT_sb = pers.tile([P, 2 * L], f32, tag="bT")
        stats = pers.tile([P, 12], f32, tag="stats")
        ac_sb = pers.tile([P, 4], f32, tag="ac")
        bnw = pers.tile([P, 8], f32, tag="bnw")
        hN = pers.tile([P, BPC * H], bf16, tag="hN")
        ident_bf = pers.tile([P, P], bf16, tag="ident")
        ones128b = pers.tile([P, 1], bf16, tag="ones128b")
        ones1 = pers.tile([1, P], f32, tag="ones1")
        w1_sb = pers.tile([P, 2 * P], f32, tag="w1")
        w2_sb = pers.tile([P, 64], f32, tag="w2")
        w3_sb = pers.tile([64, 1], f32, tag="w3")
        b1_sb = pers.tile([P, 1], f32, tag="b1")
        b2_sb = pers.tile([64, 1], f32, tag="b2")
        b3_sb = pers.tile([1, 1], f32, tag="b3")
        scal = pers.tile([1, 2 * G], f32, tag="scal")

        # ---- DRAM bounce buffers ---------------------------------------
        HPC = NPC // 2  # 640 rows per half-shard
        ag_in = dram.tile([NPC, H], bf16, tag="ag_in")
        ag_outs = [
            [dram.tile([NCORE * HPC, H], bf16, tag=f"ag_out{hh}_{ll}",
                       name=f"ag_out{hh}_{ll}", addr_space="Shared")
             for hh in range(2)]
            for ll in range(L)]
        ar_in = dram.tile([P, 4], f32, tag="ar_in")
        ar_out = dram.tile([P, 4], f32, tag="ar_out")
        pr_in = dram.tile([2 * P + 1, G], f32, tag="pr_in")
        pr_out = dram.tile([2 * P + 1, G], f32, tag="pr_out")

        # ---- input loads ------------------------------------------------
        for t, d in [(xT_sb, d_xT), (D_sb, d_D), (baseT_sb, d_baseT),
                     (nrmT_sb, d_nrmT), (W_sb, d_W), (gT_sb, d_gT),
                     (bT_sb, d_bT), (w1_sb, d_w1), (w2_sb, d_w2),
                     (w3_sb, d_w3), (b1_sb, d_b1), (b2_sb, d_b2),
                     (b3_sb, d_b3)]:
            nc.sync.dma_start(out=t[:], in_=d[:])

        make_identity(nc, ident_bf[:])
        nc.vector.memset(ones128b[:], 1.0)
        nc.vector.memset(ones1[:], 1.0)

        # nrm_rep[p, j] = nrm[j] : rank-1 broadcast via f32 matmuls
        for j0 in range(0, NPC, 512):
            j1 = min(j0 + 512, NPC)
            ps_n = psV.tile([P, 512], f32, tag="vec")
            nc.tensor.matmul(out=ps_n[:, :j1 - j0], lhsT=ones1[:],
                             rhs=nrmT_sb[:, j0:j1], start=True, stop=True)
            nc.vector.tensor_copy(out=nrm_rep[:, j0:j1], in_=ps_n[:, :j1 - j0])

        # ---- encoder: hT = baseT + D^T @ xT (per h-half) ----------------
        for half in range(2):
            for (j0, j1) in PASSES:
                ps_e = psT.tile([P, j1 - j0], f32, tag=f"spmm{half}")
                nc.tensor.matmul(
                    out=ps_e[:],
                    lhsT=D_sb[:, half * P:(half + 1) * P],
                    rhs=xT_sb[:, j0:j1], start=True, stop=True)
                hslc = hT[:, half * NPC + j0:half * NPC + j1]
                nc.vector.tensor_scalar_add(hslc, ps_e[:],
                                            baseT_sb[:, half:half + 1])
                nc.vector.tensor_tensor(
                    out=hsT[:, half * NPC + j0:half * NPC + j1],
                    in0=hslc, in1=nrm_rep[:, j0:j1], op=OP.mult)

        if variant == "enc":
            nc.sync.dma_start(out=d_out[:], in_=hT[0:1, 0:G])

        # ---- layers -----------------------------------------------------
        nlayers = 0 if variant == "enc" else (1 if variant in ("ag", "l1") else L)
        for l in range(nlayers):
            # GEMM hws[n, j] = sum_h hsT[h, n] W[l][h, j]; write bf16 table
            for nb in range(BPC):
                ps_g = psG.tile([P, H], f32, tag="mm")
                for k in range(2):
                    nc.tensor.matmul(
                        out=ps_g[:],
                        lhsT=hsT[:, k * NPC + nb * P:k * NPC + (nb + 1) * P],
                        rhs=W_sb[:, (l * 2 + k) * H:(l * 2 + k + 1) * H],
                        start=(k == 0), stop=(k == 1))
                hws_bf = work.tile([P, H], bf16, tag="hws")
                nc.vector.tensor_copy(out=hws_bf[:], in_=ps_g[:])
                nc.sync.dma_start(out=ag_in[nb * P:(nb + 1) * P, :],
                                  in_=hws_bf[:])
            nc.gpsimd.collective_compute(
                "AllGather", OP.bypass, replica_groups=rg,
                ins=[ag_in[0:HPC, :]], outs=[ag_outs[l][0][:]])
            nc.gpsimd.collective_compute(
                "AllGather", OP.bypass, replica_groups=rg,
                ins=[ag_in[HPC:NPC, :]], outs=[ag_outs[l][1][:]])
            if variant == "ag":
                sbt = work.tile([1, G], bf16, tag="dbg")
                nc.sync.dma_start(out=sbt[:], in_=ag_outs[l][0][0:1, 0:G])
                sbt2 = work.tile([1, G], f32, tag="dbg2")
                nc.vector.tensor_copy(out=sbt2[:], in_=sbt[:])
                nc.sync.dma_start(out=d_out[:], in_=sbt2[:])
                break

            # load full table into SBUF (slab slot order matches host perm)
            for h, ago in ((0, ag_outs[l][0]), (1, ag_outs[l][1])):
                for k in range(NCORE):
                    pos = (h * NCORE + k) * (BPC // 2)
                    nc.sync.dma_start(
                        out=tbl[:, pos * H:(pos + BPC // 2) * H].rearrange(
                            "p (b h2) -> p b h2", h2=H),
                        in_=ago[k * HPC:(k + 1) * HPC, :].rearrange(
                            "(b p) h2 -> p b h2", p=P))

            # SpMM: t^T[h, d] = sum_s tbl[s, h] A[s, d], 3 dst passes
            for p, (j0, j1) in enumerate(PASSES):
                ps_h = [psT.tile([P, j1 - j0], f32, tag=f"spmm{half}",
                                 name=f"ps_h{half}")
                        for half in range(2)]
                for s in range(SB):
                    A_t = stream.tile([P, j1 - j0], fp8, tag="A")
                    nc.sync.dma_start(
                        out=A_t[:],
                        in_=d_A[:, s * NPC + j0:s * NPC + j1])
                    for half in range(2):
                        nc.tensor.matmul(
                            out=ps_h[half][:],
                            lhsT=tbl[:, s * H + half * P:s * H + (half + 1) * P],
                            rhs=A_t[:], start=(s == 0), stop=(s == SB - 1))
                # nrm_dst scale + per-pass row-sum stats
                for half in range(2):
                    tslc = tT[:, half * NPC + j0:half * NPC + j1]
                    nc.vector.tensor_tensor(
                        out=tslc, in0=ps_h[half][:],
                        in1=nrm_rep[:, j0:j1], op=OP.mult)
                    col = 2 * p + half
                    nc.vector.tensor_reduce(
                        out=stats[:, col:col + 1], in_=tslc,
                        axis=mybir.AxisListType.X, op=OP.add)
                    sq = work.tile([P, j1 - j0], f32, tag="sq")
                    nc.vector.affine_mul_reduce(
                        out=sq[:], accum_out=stats[:, 6 + col:7 + col],
                        in0=tslc, in1=tslc, scale=1.0, bias=0.0)

            # combine pass partials -> [P,4] (sum0,sum1,q0,q1), AllReduce
            pack = work.tile([P, 4], f32, tag="pack")
            nc.vector.tensor_tensor(out=pack[:, 0:2], in0=stats[:, 0:2],
                                    in1=stats[:, 2:4], op=OP.add)
            nc.vector.tensor_tensor(out=pack[:, 0:2], in0=pack[:, 0:2],
                                    in1=stats[:, 4:6], op=OP.add)
            nc.vector.tensor_tensor(out=pack[:, 2:4], in0=stats[:, 6:8],
                                    in1=stats[:, 8:10], op=OP.add)
            nc.vector.tensor_tensor(out=pack[:, 2:4], in0=pack[:, 2:4],
                                    in1=stats[:, 10:12], op=OP.add)
            nc.sync.dma_start(out=ar_in[:], in_=pack[:])
            nc.gpsimd.collective_compute(
                "AllReduce", OP.add, replica_groups=rg,
                ins=[ar_in[:]], outs=[ar_out[:]])
            stv = work.tile([P, 4], f32, tag="stv")
            nc.sync.dma_start(out=stv[:], in_=ar_out[:])

            # per-partition BN: a = gamma*istd, c = beta - mu*a
            mu = bnw[:, 0:2]
            var = bnw[:, 2:4]
            msq = bnw[:, 4:6]
            nc.vector.tensor_scalar_mul(mu, stv[:, 0:2], 1.0 / N)
            nc.vector.tensor_scalar_mul(var, stv[:, 2:4], 1.0 / N)
            nc.vector.tensor_tensor(out=msq, in0=mu, in1=mu, op=OP.mult)
            nc.vector.tensor_tensor(out=var, in0=var, in1=msq, op=OP.subtract)
            nc.vector.tensor_scalar_add(var, var, BN_EPS)
            nc.vector.reciprocal(out=var, in_=var)
            nc.scalar.activation(out=var, in_=var, func=FT.Sqrt)  # istd
            nc.vector.tensor_tensor(out=ac_sb[:, 0:2], in0=var,
                                    in1=gT_sb[:, 2 * l:2 * l + 2], op=OP.mult)
            nc.vector.tensor_tensor(out=msq, in0=mu, in1=ac_sb[:, 0:2], op=OP.mult)
            nc.vector.tensor_tensor(out=ac_sb[:, 2:4],
                                    in0=bT_sb[:, 2 * l:2 * l + 2],
                                    in1=msq, op=OP.subtract)

            # apply: h += relu(a*t + c); hs = h*nrm (bf16)
            for half in range(2):
                hslc = hT[:, half * NPC:(half + 1) * NPC]
                tslc = tT[:, half * NPC:(half + 1) * NPC]
                u = work.tile([P, NPC], f32, tag="u")
                nc.scalar.activation(out=u[:], in_=tslc, func=FT.Relu,
                                     scale=ac_sb[:, half:half + 1],
                                     bias=ac_sb[:, 2 + half:3 + half])
                nc.vector.tensor_tensor(out=hslc, in0=hslc, in1=u[:], op=OP.add)
                if l < nlayers - 1 or variant in ("l1",):
                    nc.vector.tensor_tensor(
                        out=hsT[:, half * NPC:(half + 1) * NPC],
                        in0=hslc, in1=nrm_rep[:], op=OP.mult)

        if variant == "l1":
            nc.sync.dma_start(out=d_out[:], in_=hT[0:1, 0:G])
        skip_pool = variant in ("enc", "ag", "l1")

        # ---- pooling + MLP ---------------------------------------------
        if not skip_pool:
            # transpose all of hT back to node-major first (keeps the pool
            # matmul accumulation groups free of interleaved transposes)
            for nb in range(BPC):
                hb = work.tile([P, H], bf16, tag="hb")
                for half in range(2):
                    nc.vector.tensor_copy(
                        out=hb[:, half * P:(half + 1) * P],
                        in_=hT[:, half * NPC + nb * P:half * NPC + (nb + 1) * P])
                for half in range(2):
                    pst = psG.tile([P, P], bf16, tag="mm")
                    nc.tensor.transpose(out=pst[:], in_=hb[:, half * P:(half + 1) * P],
                                        identity=ident_bf[:])
                    nc.vector.tensor_copy(
                        out=hN[:, nb * H + half * P:nb * H + (half + 1) * P],
                        in_=pst[:])
            ps_p0 = psT.tile([P, G], f32, tag="spmm0")
            ps_p1 = psT.tile([P, G], f32, tag="spmm1")
            ps_pc = psV.tile([1, G], f32, tag="vec")
            for nb in range(BPC):
                psel_t = stream.tile([P, G], bf16, tag="psel")
                nc.sync.dma_start(out=psel_t[:], in_=d_psel[:, nb * G:(nb + 1) * G])
                nc.tensor.matmul(out=ps_p0[:], lhsT=hN[:, nb * H:nb * H + P],
                                 rhs=psel_t[:],
                                 start=(nb == 0), stop=(nb == BPC - 1))
                nc.tensor.matmul(out=ps_p1[:], lhsT=hN[:, nb * H + P:(nb + 1) * H],
                                 rhs=psel_t[:],
                                 start=(nb == 0), stop=(nb == BPC - 1))
                nc.tensor.matmul(out=ps_pc[:], lhsT=ones128b[:], rhs=psel_t[:],
                                 start=(nb == 0), stop=(nb == BPC - 1))
            g0 = work.tile([P, G], f32, tag="g0")
            g1 = work.tile([P, G], f32, tag="g1")
            cnt = scal[:, 0:G]
            nc.vector.tensor_copy(out=g0[:], in_=ps_p0[:])
            nc.vector.tensor_copy(out=g1[:], in_=ps_p1[:])
            nc.vector.tensor_copy(out=cnt, in_=ps_pc[:])
            nc.sync.dma_start(out=pr_in[0:P, :], in_=g0[:])
            nc.sync.dma_start(out=pr_in[P:2 * P, :], in_=g1[:])
            nc.sync.dma_start(out=pr_in[2 * P:2 * P + 1, :], in_=cnt)
            nc.gpsimd.collective_compute(
                "AllReduce", OP.add, replica_groups=rg,
                ins=[pr_in[:]], outs=[pr_out[:]])
            nc.sync.dma_start(out=g0[:], in_=pr_out[0:P, :])
            nc.sync.dma_start(out=g1[:], in_=pr_out[P:2 * P, :])
            nc.sync.dma_start(out=cnt, in_=pr_out[2 * P:2 * P + 1, :])
            nc.vector.tensor_scalar_max(cnt, cnt, 1.0)
            nc.vector.reciprocal(out=cnt, in_=cnt)
            ps_r = psV.tile([P, G], f32, tag="vec")
            nc.tensor.matmul(out=ps_r[:], lhsT=ones1[:], rhs=cnt, start=True, stop=True)
            rc_rep = work.tile([P, G], f32, tag="rc_rep")
            nc.vector.tensor_copy(out=rc_rep[:], in_=ps_r[:])
            nc.vector.tensor_tensor(out=g0[:], in0=g0[:], in1=rc_rep[:], op=OP.mult)
            nc.vector.tensor_tensor(out=g1[:], in0=g1[:], in1=rc_rep[:], op=OP.mult)

            # MLP head (weights as lhsT, graphs along free dim)
            ps1 = psV.tile([P, G], f32, tag="vec")
            nc.tensor.matmul(out=ps1[:], lhsT=w1_sb[:, 0:P], rhs=g0[:], start=True, stop=False)
            nc.tensor.matmul(out=ps1[:], lhsT=w1_sb[:, P:2 * P], rhs=g1[:], start=False, stop=True)
            y1 = work.tile([P, G], f32, tag="y1")
            nc.scalar.activation(out=y1[:], in_=ps1[:], func=FT.Relu, bias=b1_sb[:, 0:1])
            ps2 = psV.tile([64, G], f32, tag="vec")
            nc.tensor.matmul(out=ps2[:], lhsT=w2_sb[:], rhs=y1[:], start=True, stop=True)
            y2 = work.tile([64, G], f32, tag="y2")
            nc.scalar.activation(out=y2[:], in_=ps2[:], func=FT.Relu, bias=b2_sb[:, 0:1])
            ps3 = psV.tile([1, G], f32, tag="vec")
            nc.tensor.matmul(out=ps3[:], lhsT=w3_sb[:], rhs=y2[:], start=True, stop=True)
            y3 = work.tile([1, G], f32, tag="y3")
            nc.vector.tensor_scalar_add(y3[:], ps3[:], b3_sb[0:1, 0:1])
            nc.sync.dma_start(out=d_out[:], in_=y3[:])

    nc.compile()
    return nc


# --------------------------------------------------------------------------
# entry point
# --------------------------------------------------------------------------

def kernel(x, edge_index, batch_ids, emb, W, b, gamma, beta,
           mlp_W1, mlp_b1, mlp_W2, mlp_b2, mlp_W3, mlp_b3,
           _trace=False, _trace_kwargs=None):
    # NB: reference BN subtracts the per-channel mean, so the additive bias b
    # cancels exactly and is not needed by the device program.
    in_maps = _preprocess(x, edge_index, batch_ids, emb, W, gamma, beta,
                          mlp_W1, mlp_b1, mlp_W2, mlp_b2, mlp_W3, mlp_b3)
    import os
    variant = os.environ.get("KVARIANT", "full")
    if variant not in _compiled:
        _compiled[variant] = _build(variant)
    nc = _compiled[variant]
    kw = {}
    if _trace:
        kw = dict(trace=True, **(_trace_kwargs or {}))
    res = run_bass_kernel_spmd(nc, in_maps, core_ids=list(range(NCORE)), **kw)
    out = np.asarray(res.results[0]["out"], np.float32).reshape(G, 1)
    kernel._last_results = res
    return out


# revision 16
# speedup vs baseline: 2.7119x; 1.6919x over previous
"""Trainium2 Bass kernel for HIVNet GCN message passing (8-core SPMD).

V2 design (replaces dma_gather-based V1 whose GpSimd descriptor generation
was the bottleneck at ~18us per gather call, 1.45ms total):

  - h kept TRANSPOSED on-chip: hT [128 (h-half), 2 x 1280 nodes] f32.
  - Per layer: GEMM hws = (h*nrm) @ W[l] via lhsT = hsT blocks (no explicit
    transposes needed), result written bf16 to a DRAM table; AllGather with
    SHARED output (each core writes only its 655KB slice).
  - Edge aggregation as block-dense SpMM on TensorE: t^T[h, dst] =
    sum_s table[s, h] * A[s, dst], where A is the host-built [10240 x 1280]
    dst-shard adjacency-count matrix (self loops included), streamed from
    DRAM as fp8e4 (counts are small integers => exact). 320 matmuls/layer,
    ~85us PE time, zero GpSimd work.
  - BN stats via DVE tensor_tensor_reduce (fused nrm_dst scaling + row sums),
    2KB AllReduce, per-partition scale/bias applied with one ACT op per half.
  - Readout: transpose final h blocks back to node-major, pool one-hot
    matmuls, 257-row AllReduce, 3-layer MLP.
"""

import sys

sys.path.insert(0, "/opt/trn_rl_repo")

from contextlib import ExitStack

import numpy as np
import ml_dtypes

from concourse import bass, mybir, bacc, tile
from concourse.bass_utils import run_bass_kernel_spmd
from concourse.masks import make_identity

NCORE = 8
P = 128
H = 256
L = 4
NF = 9
G = 256
N = 10000
BPC = 10                # dst blocks per core
NPC = BPC * P           # 1280 nodes per core
NPAD = NCORE * NPC      # 10240
SB = NPAD // P          # 80 src blocks
PASSES = [(0, 512), (512, 1024), (1024, 1280)]  # dst column passes (<=512)
BN_EPS = 1e-5

f32 = mybir.dt.float32
bf16 = mybir.dt.bfloat16
fp8 = mybir.dt.float8e4
bfnp = ml_dtypes.bfloat16
fp8np = ml_dtypes.float8_e4m3fn

FT = mybir.ActivationFunctionType
OP = mybir.AluOpType

_compiled = {}


# --------------------------------------------------------------------------
# host-side structural preprocessing
# --------------------------------------------------------------------------

def _preprocess(x, edge_index, batch_ids, emb, W, gamma, beta,
                mlp_W1, mlp_b1, mlp_W2, mlp_b2, mlp_W3, mlp_b3):
    src = np.asarray(edge_index[0], np.int64)
    dst = np.asarray(edge_index[1], np.int64)

    deg = np.zeros(NPAD, np.float64)
    np.add.at(deg, dst, 1.0)
    deg[:N] += 1.0  # self loops
    nrm = np.zeros(NPAD, np.float32)
    nrm[:N] = 1.0 / np.sqrt(deg[:N])

    # per-core adjacency count matrices A_c [NPAD, NPC], fp8-exact ints
    order = np.argsort(dst, kind="stable")
    s_sorted = src[order]
    d_sorted = dst[order]
    bounds = np.searchsorted(d_sorted, np.arange(NCORE + 1) * NPC)

    x_np = np.zeros((NPAD, NF), np.float32)
    x_np[:N] = np.asarray(x, np.float64)

    bids = np.asarray(batch_ids, np.int64)
    psel_full = np.zeros((NPAD, G), np.float32)
    psel_full[np.arange(N), bids] = 1.0

    embf = np.asarray(emb, np.float32)
    emb0 = np.ascontiguousarray(embf[:, 0, :])                # [NF, H]
    D = np.ascontiguousarray(embf[:, 1, :] - embf[:, 0, :])   # [NF, H]
    base = emb0.sum(axis=0)                                   # [H]

    Wf = np.asarray(W, np.float32)                            # [L,H,H]
    W_r = Wf.reshape(L, 2, P, H).transpose(2, 0, 1, 3).reshape(P, L * 2 * H)

    gT = np.asarray(gamma, np.float32).reshape(L * 2, P).T    # [P, 2L]
    bT = np.asarray(beta, np.float32).reshape(L * 2, P).T     # [P, 2L]
    baseT = base.reshape(2, P).T                              # [P, 2]

    w1 = np.asarray(mlp_W1, np.float32).reshape(2, P, P).transpose(1, 0, 2).reshape(P, 2 * P)
    w2 = np.asarray(mlp_W2, np.float32)                       # [128,64]
    w3 = np.asarray(mlp_W3, np.float32)                       # [64,1]
    b1 = np.asarray(mlp_b1, np.float32).reshape(P, 1)
    b2 = np.asarray(mlp_b2, np.float32).reshape(64, 1)
    b3 = np.asarray(mlp_b3, np.float32).reshape(1, 1)

    in_maps = []
    for c in range(NCORE):
        lo, hi = c * NPC, (c + 1) * NPC

        es, ee = bounds[c], bounds[c + 1]
        a_idx = s_sorted[es:ee] * np.int64(NPC) + (d_sorted[es:ee] - lo)
        A = np.bincount(a_idx, minlength=NPAD * NPC).astype(np.float32)
        A = A.reshape(NPAD, NPC)
        own = np.arange(lo, min(hi, N), dtype=np.int64)
        A[own, own - lo] += 1.0  # self loops
        # src-block order: all cores' first half-shards, then second halves,
        # so SpMM on the first 40 slab slots only needs the first AllGather.
        perm = [k * BPC + h * (BPC // 2) + bb
                for h in range(2) for k in range(NCORE) for bb in range(BPC // 2)]
        A_blk = np.ascontiguousarray(
            A.reshape(SB, P, NPC)[perm].transpose(1, 0, 2)).reshape(P, SB * NPC)

        nrmT = np.ascontiguousarray(nrm[lo:hi]).reshape(1, NPC)
        xTc = np.ascontiguousarray(x_np[lo:hi].T)             # [NF, NPC]

        pselc = psel_full[lo:hi].reshape(BPC, P, G)
        pselc = np.ascontiguousarray(pselc.transpose(1, 0, 2)).reshape(P, BPC * G)

        in_maps.append(dict(
            A=A_blk.astype(fp8np), xT=xTc, nrmT=nrmT,
            D=D, baseT=baseT, W=W_r.astype(bfnp), gT=gT, bT=bT,
            psel=pselc.astype(bfnp),
            w1=w1, w2=w2, w3=w3, b1=b1, b2=b2, b3=b3,
        ))
    return in_maps


# --------------------------------------------------------------------------
# device program
# --------------------------------------------------------------------------

def _build(variant="full"):
    nc = bacc.Bacc(None, target_bir_lowering=False)

    d_A = nc.dram_tensor("A", [P, SB * NPC], fp8, kind="ExternalInput")
    d_xT = nc.dram_tensor("xT", [NF, NPC], f32, kind="ExternalInput")
    d_nrmT = nc.dram_tensor("nrmT", [1, NPC], f32, kind="ExternalInput")
    d_D = nc.dram_tensor("D", [NF, H], f32, kind="ExternalInput")
    d_baseT = nc.dram_tensor("baseT", [P, 2], f32, kind="ExternalInput")
    d_W = nc.dram_tensor("W", [P, L * 2 * H], bf16, kind="ExternalInput")
    d_gT = nc.dram_tensor("gT", [P, 2 * L], f32, kind="ExternalInput")
    d_bT = nc.dram_tensor("bT", [P, 2 * L], f32, kind="ExternalInput")
    d_psel = nc.dram_tensor("psel", [P, BPC * G], bf16, kind="ExternalInput")
    d_w1 = nc.dram_tensor("w1", [P, 2 * P], f32, kind="ExternalInput")
    d_w2 = nc.dram_tensor("w2", [P, 64], f32, kind="ExternalInput")
    d_w3 = nc.dram_tensor("w3", [64, 1], f32, kind="ExternalInput")
    d_b1 = nc.dram_tensor("b1", [P, 1], f32, kind="ExternalInput")
    d_b2 = nc.dram_tensor("b2", [64, 1], f32, kind="ExternalInput")
    d_b3 = nc.dram_tensor("b3", [1, 1], f32, kind="ExternalInput")
    d_out = nc.dram_tensor("out", [1, G], f32, kind="ExternalOutput")

    rg = [list(range(NCORE))]

    with tile.TileContext(nc) as tc, ExitStack() as ctx:
        pers = ctx.enter_context(tc.tile_pool(name="pers", bufs=1))
        psT = ctx.enter_context(tc.tile_pool(name="psT", bufs=1, space="PSUM"))
        psG = ctx.enter_context(tc.tile_pool(name="psG", bufs=1, space="PSUM"))
        psV = ctx.enter_context(tc.tile_pool(name="psV", bufs=1, space="PSUM"))
        stream = ctx.enter_context(tc.tile_pool(name="stream", bufs=3))
        work = ctx.enter_context(tc.tile_pool(name="work", bufs=2))
        dram = ctx.enter_context(tc.tile_pool(name="dram", bufs=1, space="DRAM"))

        # ---- persistent SBUF state -------------------------------------
        hT = pers.tile([P, 2 * NPC], f32, tag="hT")
        hsT = pers.tile([P, 2 * NPC], bf16, tag="hsT")
        tT = pers.tile([P, 2 * NPC], f32, tag="tT")
        nrm_rep = pers.tile([P, NPC], f32, tag="nrm_rep")
        A_sb = pers.tile([P, SB * NPC], fp8, tag="A_sb")
        xT_sb = pers.tile([NF, NPC], f32, tag="xT")
        D_sb = pers.tile([NF, H], f32, tag="D")
        baseT_sb = pers.tile([P, 2], f32, tag="baseT")
        nrmT_sb = pers.tile([1, NPC], f32, tag="nrmT")
        W_sb = pers.tile([P, L * 2 * H], bf16, tag="W")
        gT_sb = pers.tile([P, 2 * L], f32, tag="gT")
        bT_sb = pers.tile([P, 2 * L], f32, tag="bT")
        stats = pers.tile([P, 12], f32, tag="stats")
        ac_sb = pers.tile([P, 4], f32, tag="ac")
        bnw = pers.tile([P, 8], f32, tag="bnw")
        hN = pers.tile([P, BPC * H], bf16, tag="hN")
        ident_bf = pers.tile([P, P], bf16, tag="ident")
        ones128b = pers.tile([P, 1], bf16, tag="ones128b")
        ones1 = pers.tile([1, P], f32, tag="ones1")
        w1_sb = pers.tile([P, 2 * P], f32, tag="w1")
        w2_sb = pers.tile([P, 64], f32, tag="w2")
        w3_sb = pers.tile([64, 1], f32, tag="w3")
        b1_sb = pers.tile([P, 1], f32, tag="b1")
        b2_sb = pers.tile([64, 1], f32, tag="b2")
        b3_sb = pers.tile([1, 1], f32, tag="b3")
        scal = pers.tile([1, 2 * G], f32, tag="scal")

        # ---- DRAM bounce buffers ---------------------------------------
        HPC = NPC // 2  # 640 rows per half-shard
        ag_in = dram.tile([NPC, H], bf16, tag="ag_in")
        ag_outs = [
            [dram.tile([NCORE * HPC, H], bf16, tag=f"ag_out{hh}_{ll}",
                       name=f"ag_out{hh}_{ll}", addr_space="Shared")
             for hh in range(2)]
            for ll in range(L)]
        ar_in = dram.tile([P, 4], f32, tag="ar_in")
        ar_out = dram.tile([P, 4], f32, tag="ar_out")
        pr_in = dram.tile([2 * P + 1, G], f32, tag="pr_in")
        pr_out = dram.tile([2 * P + 1, G], f32, tag="pr_out")

        # ---- input loads ------------------------------------------------
        for t, d in [(xT_sb, d_xT), (D_sb, d_D), (baseT_sb, d_baseT),
                     (nrmT_sb, d_nrmT), (W_sb, d_W), (gT_sb, d_gT),
                     (bT_sb, d_bT), (w1_sb, d_w1), (w2_sb, d_w2),
                     (w3_sb, d_w3), (b1_sb, d_b1), (b2_sb, d_b2),
                     (b3_sb, d_b3)]:
            nc.sync.dma_start(out=t[:], in_=d[:])

        ACH = SB // 16  # 5 src blocks per A-load chunk
        for ch in range(16):
            nc.sync.dma_start(
                out=A_sb[:, ch * ACH * NPC:(ch + 1) * ACH * NPC],
                in_=d_A[:, ch * ACH * NPC:(ch + 1) * ACH * NPC])

        make_identity(nc, ident_bf[:])
        nc.vector.memset(ones128b[:], 1.0)
        nc.vector.memset(ones1[:], 1.0)

        # nrm_rep[p, j] = nrm[j] : rank-1 broadcast via f32 matmuls
        for j0 in range(0, NPC, 512):
            j1 = min(j0 + 512, NPC)
            ps_n = psV.tile([P, 512], f32, tag="vec")
            nc.tensor.matmul(out=ps_n[:, :j1 - j0], lhsT=ones1[:],
                             rhs=nrmT_sb[:, j0:j1], start=True, stop=True)
            nc.vector.tensor_copy(out=nrm_rep[:, j0:j1], in_=ps_n[:, :j1 - j0])

        # ---- encoder: hT = baseT + D^T @ xT (per h-half) ----------------
        for half in range(2):
            for p, (j0, j1) in enumerate(PASSES):
                ps_e = psT.tile([P, j1 - j0], f32, tag=f"sp{half}{p}",
                                name="ps_e")
                pe_sl = ps_e[:]
                nc.tensor.matmul(
                    out=pe_sl,
                    lhsT=D_sb[:, half * P:(half + 1) * P],
                    rhs=xT_sb[:, j0:j1], start=True, stop=True)
                hslc = hT[:, half * NPC + j0:half * NPC + j1]
                nc.vector.tensor_scalar_add(hslc, pe_sl,
                                            baseT_sb[:, half:half + 1])
                nc.vector.tensor_tensor(
                    out=hsT[:, half * NPC + j0:half * NPC + j1],
                    in0=hslc, in1=nrm_rep[:, j0:j1], op=OP.mult)

        if variant == "enc":
            nc.sync.dma_start(out=d_out[:], in_=hT[0:1, 0:G])

        # ---- layers -----------------------------------------------------
        nlayers = 0 if variant == "enc" else (1 if variant in ("ag", "l1") else L)
        for l in range(nlayers):
            # GEMM hws[n, j] = sum_h hsT[h, n] W[l][h, j]; write bf16 table
            for nb in range(BPC):
                ps_g = psG.tile([P, H], f32, tag="mm")
                for k in range(2):
                    nc.tensor.matmul(
                        out=ps_g[:],
                        lhsT=hsT[:, k * NPC + nb * P:k * NPC + (nb + 1) * P],
                        rhs=W_sb[:, (l * 2 + k) * H:(l * 2 + k + 1) * H],
                        start=(k == 0), stop=(k == 1))
                hws_bf = work.tile([P, H], bf16, tag="hws")
                nc.vector.tensor_copy(out=hws_bf[:], in_=ps_g[:])
                nc.sync.dma_start(out=ag_in[nb * P:(nb + 1) * P, :],
                                  in_=hws_bf[:])
            nc.gpsimd.collective_compute(
                "AllGather", OP.bypass, replica_groups=rg,
                ins=[ag_in[0:HPC, :]], outs=[ag_outs[l][0][:]])
            nc.gpsimd.collective_compute(
                "AllGather", OP.bypass, replica_groups=rg,
                ins=[ag_in[HPC:NPC, :]], outs=[ag_outs[l][1][:]])
            if variant == "ag":
                sbt = work.tile([1, G], bf16, tag="dbg")
                nc.sync.dma_start(out=sbt[:], in_=ag_outs[l][0][0:1, 0:G])
                sbt2 = work.tile([1, G], f32, tag="dbg2")
                nc.vector.tensor_copy(out=sbt2[:], in_=sbt[:])
                nc.sync.dma_start(out=d_out[:], in_=sbt2[:])
                break

            # stream table chunks (5 src blocks each) from the shared AG
            # buffers; slab slot order matches the host-side perm
            tch_list = []
            for h, ago in ((0, ag_outs[l][0]), (1, ag_outs[l][1])):
                for k in range(NCORE):
                    tch = stream.tile([P, (BPC // 2) * H], bf16, tag="tbl",
                                      bufs=8, name="tch")
                    nc.sync.dma_start(
                        out=tch[:].rearrange("p (b h2) -> p b h2", h2=H),
                        in_=ago[k * HPC:(k + 1) * HPC, :].rearrange(
                            "(b p) h2 -> p b h2", p=P))
                    tch_list.append(tch)

            # SpMM: t^T[h, d] = sum_s tbl[s, h] A[s, d]; s-major with all
            # six (half x pass) PSUM accumulators open
            ps_a = {}
            for half in range(2):
                for p, (j0, j1) in enumerate(PASSES):
                    ps_a[(half, p)] = psT.tile(
                        [P, j1 - j0], f32, tag=f"sp{half}{p}", name="ps_acc")[:]
            for s in range(SB):
                tch = tch_list[s // (BPC // 2)]
                off = (s % (BPC // 2)) * H
                for half in range(2):
                    lhsT = tch[:, off + half * P:off + (half + 1) * P]
                    for p, (j0, j1) in enumerate(PASSES):
                        nc.tensor.matmul(
                            out=ps_a[(half, p)],
                            lhsT=lhsT,
                            rhs=A_sb[:, s * NPC + j0:s * NPC + j1],
                            start=(s == 0), stop=(s == SB - 1))
            # nrm_dst scale + row-sum stats
            for half in range(2):
                for p, (j0, j1) in enumerate(PASSES):
                    tslc = tT[:, half * NPC + j0:half * NPC + j1]
                    nc.vector.tensor_tensor(
                        out=tslc, in0=ps_a[(half, p)],
                        in1=nrm_rep[:, j0:j1], op=OP.mult)
                    col = 2 * p + half
                    nc.vector.tensor_reduce(
                        out=stats[:, col:col + 1], in_=tslc,
                        axis=mybir.AxisListType.X, op=OP.add)
                    sq = work.tile([P, j1 - j0], f32, tag="sq")
                    nc.vector.affine_mul_reduce(
                        out=sq[:], accum_out=stats[:, 6 + col:7 + col],
                        in0=tslc, in1=tslc, scale=1.0, bias=0.0)

            # combine pass partials -> [P,4] (sum0,sum1,q0,q1), AllReduce
            pack = work.tile([P, 4], f32, tag="pack")
            nc.vector.tensor_tensor(out=pack[:, 0:2], in0=stats[:, 0:2],
                                    in1=stats[:, 2:4], op=OP.add)
            nc.vector.tensor_tensor(out=pack[:, 0:2], in0=pack[:, 0:2],
                                    in1=stats[:, 4:6], op=OP.add)
            nc.vector.tensor_tensor(out=pack[:, 2:4], in0=stats[:, 6:8],
                                    in1=stats[:, 8:10], op=OP.add)
            nc.vector.tensor_tensor(out=pack[:, 2:4], in0=pack[:, 2:4],
                                    in1=stats[:, 10:12], op=OP.add)
            nc.sync.dma_start(out=ar_in[:], in_=pack[:])
            nc.gpsimd.collective_compute(
                "AllReduce", OP.add, replica_groups=rg,
                ins=[ar_in[:]], outs=[ar_out[:]])
            stv = work.tile([P, 4], f32, tag="stv")
            nc.sync.dma_start(out=stv[:], in_=ar_out[:])

            # per-partition BN: a = gamma*istd, c = beta - mu*a
            mu = bnw[:, 0:2]
            var = bnw[:, 2:4]
            msq = bnw[:, 4:6]
            nc.vector.tensor_scalar_mul(mu, stv[:, 0:2], 1.0 / N)
            nc.vector.tensor_scalar_mul(var, stv[:, 2:4], 1.0 / N)
            nc.vector.tensor_tensor(out=msq, in0=mu, in1=mu, op=OP.mult)
            nc.vector.tensor_tensor(out=var, in0=var, in1=msq, op=OP.subtract)
            nc.vector.tensor_scalar_add(var, var, BN_EPS)
            nc.vector.reciprocal(out=var, in_=var)
            nc.scalar.activation(out=var, in_=var, func=FT.Sqrt)  # istd
            nc.vector.tensor_tensor(out=ac_sb[:, 0:2], in0=var,
                                    in1=gT_sb[:, 2 * l:2 * l + 2], op=OP.mult)
            nc.vector.tensor_tensor(out=msq, in0=mu, in1=ac_sb[:, 0:2], op=OP.mult)
            nc.vector.tensor_tensor(out=ac_sb[:, 2:4],
                                    in0=bT_sb[:, 2 * l:2 * l + 2],
                                    in1=msq, op=OP.subtract)

            # apply: h += relu(a*t + c); hs = h*nrm (bf16)
            for half in range(2):
                hslc = hT[:, half * NPC:(half + 1) * NPC]
                tslc = tT[:, half * NPC:(half + 1) * NPC]
                u = work.tile([P, NPC], f32, tag="u")
                nc.scalar.activation(out=u[:], in_=tslc, func=FT.Relu,
                                     scale=ac_sb[:, half:half + 1],
                                     bias=ac_sb[:, 2 + half:3 + half])
                nc.vector.tensor_tensor(out=hslc, in0=hslc, in1=u[:], op=OP.add)
                if l < nlayers - 1 or variant in ("l1",):
                    nc.vector.tensor_tensor(
                        out=hsT[:, half * NPC:(half + 1) * NPC],
                        in0=hslc, in1=nrm_rep[:], op=OP.mult)

        if variant == "l1":
            nc.sync.dma_start(out=d_out[:], in_=hT[0:1, 0:G])
        skip_pool = variant in ("enc", "ag", "l1")

        # ---- pooling + MLP ---------------------------------------------
        if not skip_pool:
            # transpose all of hT back to node-major first (keeps the pool
            # matmul accumulation groups free of interleaved transposes)
            for nb in range(BPC):
                hb = work.tile([P, H], bf16, tag="hb")
                for half in range(2):
                    nc.vector.tensor_copy(
                        out=hb[:, half * P:(half + 1) * P],
                        in_=hT[:, half * NPC + nb * P:half * NPC + (nb + 1) * P])
                for half in range(2):
                    pst = psG.tile([P, P], bf16, tag="mm")
                    nc.tensor.transpose(out=pst[:], in_=hb[:, half * P:(half + 1) * P],
                                        identity=ident_bf[:])
                    nc.vector.tensor_copy(
                        out=hN[:, nb * H + half * P:nb * H + (half + 1) * P],
                        in_=pst[:])
            ps_p0 = psT.tile([P, G], f32, tag="sp00")
            ps_p1 = psT.tile([P, G], f32, tag="sp10")
            ps_pc = psV.tile([1, G], f32, tag="vec")
            for nb in range(BPC):
                psel_t = stream.tile([P, G], bf16, tag="psel")
                nc.sync.dma_start(out=psel_t[:], in_=d_psel[:, nb * G:(nb + 1) * G])
                nc.tensor.matmul(out=ps_p0[:], lhsT=hN[:, nb * H:nb * H + P],
                                 rhs=psel_t[:],
                                 start=(nb == 0), stop=(nb == BPC - 1))
                nc.tensor.matmul(out=ps_p1[:], lhsT=hN[:, nb * H + P:(nb + 1) * H],
                                 rhs=psel_t[:],
                                 start=(nb == 0), stop=(nb == BPC - 1))
                nc.tensor.matmul(out=ps_pc[:], lhsT=ones128b[:], rhs=psel_t[:],
                                 start=(nb == 0), stop=(nb == BPC - 1))
            g0 = work.tile([P, G], f32, tag="g0")
            g1 = work.tile([P, G], f32, tag="g1")
            cnt = scal[:, 0:G]
            nc.vector.tensor_copy(out=g0[:], in_=ps_p0[:])
            nc.vector.tensor_copy(out=g1[:], in_=ps_p1[:])
            nc.vector.tensor_copy(out=cnt, in_=ps_pc[:])
            nc.sync.dma_start(out=pr_in[0:P, :], in_=g0[:])
            nc.sync.dma_start(out=pr_in[P:2 * P, :], in_=g1[:])
            nc.sync.dma_start(out=pr_in[2 * P:2 * P + 1, :], in_=cnt)
            nc.gpsimd.collective_compute(
                "AllReduce", OP.add, replica_groups=rg,
                ins=[pr_in[:]], outs=[pr_out[:]])
            nc.sync.dma_start(out=g0[:], in_=pr_out[0:P, :])
            nc.sync.dma_start(out=g1[:], in_=pr_out[P:2 * P, :])
            nc.sync.dma_start(out=cnt, in_=pr_out[2 * P:2 * P + 1, :])
            nc.vector.tensor_scalar_max(cnt, cnt, 1.0)
            nc.vector.reciprocal(out=cnt, in_=cnt)
            ps_r = psV.tile([P, G], f32, tag="vec")
            nc.tensor.matmul(out=ps_r[:], lhsT=ones1[:], rhs=cnt, start=True, stop=True)
            rc_rep = work.tile([P, G], f32, tag="rc_rep")
            nc.vector.tensor_copy(out=rc_rep[:], in_=ps_r[:])
            nc.vector.tensor_tensor(out=g0[:], in0=g0[:], in1=rc_rep[:], op=OP.mult)
            nc.vector.tensor_tensor(out=g1[:], in0=g1[:], in1=rc_rep[:], op=OP.mult)

            # MLP head (weights as lhsT, graphs along free dim)
            ps1 = psV.tile([P, G], f32, tag="vec")
            nc.tensor.matmul(out=ps1[:], lhsT=w1_sb[:, 0:P], rhs=g0[:], start=True, stop=False)
            nc.tensor.matmul(out=ps1[:], lhsT=w1_sb[:, P:2 * P], rhs=g1[:], start=False, stop=True)
            y1 = work.tile([P, G], f32, tag="y1")
            nc.scalar.activation(out=y1[:], in_=ps1[:], func=FT.Relu, bias=b1_sb[:, 0:1])
            ps2 = psV.tile([64, G], f32, tag="vec")
            nc.tensor.matmul(out=ps2[:], lhsT=w2_sb[:], rhs=y1[:], start=True, stop=True)
            y2 = work.tile([64, G], f32, tag="y2")
            nc.scalar.activation(out=y2[:], in_=ps2[:], func=FT.Relu, bias=b2_sb[:, 0:1])
            ps3 = psV.tile([1, G], f32, tag="vec")
            nc.tensor.matmul(out=ps3[:], lhsT=w3_sb[:], rhs=y2[:], start=True, stop=True)
            y3 = work.tile([1, G], f32, tag="y3")
            nc.vector.tensor_scalar_add(y3[:], ps3[:], b3_sb[0:1, 0:1])
            nc.sync.dma_start(out=d_out[:], in_=y3[:])

    nc.compile()
    return nc


# --------------------------------------------------------------------------
# entry point
# --------------------------------------------------------------------------

def kernel(x, edge_index, batch_ids, emb, W, b, gamma, beta,
           mlp_W1, mlp_b1, mlp_W2, mlp_b2, mlp_W3, mlp_b3,
           _trace=False, _trace_kwargs=None):
    # NB: reference BN subtracts the per-channel mean, so the additive bias b
    # cancels exactly and is not needed by the device program.
    in_maps = _preprocess(x, edge_index, batch_ids, emb, W, gamma, beta,
                          mlp_W1, mlp_b1, mlp_W2, mlp_b2, mlp_W3, mlp_b3)
    import os
    variant = os.environ.get("KVARIANT", "full")
    if variant not in _compiled:
        _compiled[variant] = _build(variant)
    nc = _compiled[variant]
    kw = {}
    if _trace:
        kw = dict(trace=True, **(_trace_kwargs or {}))
    res = run_bass_kernel_spmd(nc, in_maps, core_ids=list(range(NCORE)), **kw)
    out = np.asarray(res.results[0]["out"], np.float32).reshape(G, 1)
    kernel._last_results = res
    return out


# revision 19
# speedup vs baseline: 2.9922x; 1.1034x over previous
"""Trainium2 Bass kernel for HIVNet GCN message passing (8-core SPMD).

V2 design (replaces dma_gather-based V1 whose GpSimd descriptor generation
was the bottleneck at ~18us per gather call, 1.45ms total):

  - h kept TRANSPOSED on-chip: hT [128 (h-half), 2 x 1280 nodes] f32.
  - Per layer: GEMM hws = (h*nrm) @ W[l] via lhsT = hsT blocks (no explicit
    transposes needed), result written bf16 to a DRAM table; AllGather with
    SHARED output (each core writes only its 655KB slice).
  - Edge aggregation as block-dense SpMM on TensorE: t^T[h, dst] =
    sum_s table[s, h] * A[s, dst], where A is the host-built [10240 x 1280]
    dst-shard adjacency-count matrix (self loops included), streamed from
    DRAM as fp8e4 (counts are small integers => exact). 320 matmuls/layer,
    ~85us PE time, zero GpSimd work.
  - BN stats via DVE tensor_tensor_reduce (fused nrm_dst scaling + row sums),
    2KB AllReduce, per-partition scale/bias applied with one ACT op per half.
  - Readout: transpose final h blocks back to node-major, pool one-hot
    matmuls, 257-row AllReduce, 3-layer MLP.
"""

import sys

sys.path.insert(0, "/opt/trn_rl_repo")

from contextlib import ExitStack

import numpy as np
import ml_dtypes

from concourse import bass, mybir, bacc, tile
from concourse.bass_utils import run_bass_kernel_spmd
from concourse.masks import make_identity

NCORE = 8
P = 128
H = 256
L = 4
NF = 9
G = 256
N = 10000
BPC = 10                # dst blocks per core
NPC = BPC * P           # 1280 nodes per core
NPAD = NCORE * NPC      # 10240
SB = NPAD // P          # 80 src blocks
PASSES = [(0, 512), (512, 1024), (1024, 1280)]  # dst column passes (<=512)
BN_EPS = 1e-5

f32 = mybir.dt.float32
bf16 = mybir.dt.bfloat16
fp8 = mybir.dt.float8e4
bfnp = ml_dtypes.bfloat16
fp8np = ml_dtypes.float8_e4m3fn

FT = mybir.ActivationFunctionType
OP = mybir.AluOpType

_compiled = {}


# --------------------------------------------------------------------------
# host-side structural preprocessing
# --------------------------------------------------------------------------

def _preprocess(x, edge_index, batch_ids, emb, W, gamma, beta,
                mlp_W1, mlp_b1, mlp_W2, mlp_b2, mlp_W3, mlp_b3):
    src = np.asarray(edge_index[0], np.int64)
    dst = np.asarray(edge_index[1], np.int64)

    deg = np.zeros(NPAD, np.float64)
    np.add.at(deg, dst, 1.0)
    deg[:N] += 1.0  # self loops
    nrm = np.zeros(NPAD, np.float32)
    nrm[:N] = 1.0 / np.sqrt(deg[:N])

    # per-core adjacency count matrices A_c [NPAD, NPC], fp8-exact ints
    order = np.argsort(dst, kind="stable")
    s_sorted = src[order]
    d_sorted = dst[order]
    bounds = np.searchsorted(d_sorted, np.arange(NCORE + 1) * NPC)

    x_np = np.zeros((NPAD, NF), np.float32)
    x_np[:N] = np.asarray(x, np.float64)

    bids = np.asarray(batch_ids, np.int64)
    psel_full = np.zeros((NPAD, G), np.float32)
    psel_full[np.arange(N), bids] = 1.0

    embf = np.asarray(emb, np.float32)
    emb0 = np.ascontiguousarray(embf[:, 0, :])                # [NF, H]
    D = np.ascontiguousarray(embf[:, 1, :] - embf[:, 0, :])   # [NF, H]
    base = emb0.sum(axis=0)                                   # [H]

    Wf = np.asarray(W, np.float32)                            # [L,H,H]
    W_r = Wf.reshape(L, 2, P, H).transpose(2, 0, 1, 3).reshape(P, L * 2 * H)

    gT = np.asarray(gamma, np.float32).reshape(L * 2, P).T    # [P, 2L]
    bT = np.asarray(beta, np.float32).reshape(L * 2, P).T     # [P, 2L]
    baseT = base.reshape(2, P).T                              # [P, 2]

    w1 = np.asarray(mlp_W1, np.float32).reshape(2, P, P).transpose(1, 0, 2).reshape(P, 2 * P)
    w2 = np.asarray(mlp_W2, np.float32)                       # [128,64]
    w3 = np.asarray(mlp_W3, np.float32)                       # [64,1]
    b1 = np.asarray(mlp_b1, np.float32).reshape(P, 1)
    b2 = np.asarray(mlp_b2, np.float32).reshape(64, 1)
    b3 = np.asarray(mlp_b3, np.float32).reshape(1, 1)

    in_maps = []
    for c in range(NCORE):
        lo, hi = c * NPC, (c + 1) * NPC

        es, ee = bounds[c], bounds[c + 1]
        a_idx = s_sorted[es:ee] * np.int64(NPC) + (d_sorted[es:ee] - lo)
        A = np.bincount(a_idx, minlength=NPAD * NPC).astype(np.float32)
        A = A.reshape(NPAD, NPC)
        own = np.arange(lo, min(hi, N), dtype=np.int64)
        A[own, own - lo] += 1.0  # self loops
        # src-block order: all cores' first half-shards, then second halves,
        # so SpMM on the first 40 slab slots only needs the first AllGather.
        perm = [k * BPC + h * (BPC // 2) + bb
                for h in range(2) for k in range(NCORE) for bb in range(BPC // 2)]
        A_blk = np.ascontiguousarray(
            A.reshape(SB, P, NPC)[perm].transpose(1, 0, 2)).reshape(P, SB * NPC)

        nrmT = np.ascontiguousarray(nrm[lo:hi]).reshape(1, NPC)
        xTc = np.ascontiguousarray(x_np[lo:hi].T)             # [NF, NPC]

        pselc = psel_full[lo:hi].reshape(BPC, P, G)
        pselc = np.ascontiguousarray(pselc.transpose(1, 0, 2)).reshape(P, BPC * G)

        in_maps.append(dict(
            A=A_blk.astype(fp8np), xT=xTc, nrmT=nrmT,
            D=D, baseT=baseT, W=W_r.astype(bfnp), gT=gT, bT=bT,
            psel=pselc.astype(bfnp),
            w1=w1, w2=w2, w3=w3, b1=b1, b2=b2, b3=b3,
        ))
    return in_maps


# --------------------------------------------------------------------------
# device program
# --------------------------------------------------------------------------

def _build(variant="full"):
    nc = bacc.Bacc(None, target_bir_lowering=False)

    d_A = nc.dram_tensor("A", [P, SB * NPC], fp8, kind="ExternalInput")
    d_xT = nc.dram_tensor("xT", [NF, NPC], f32, kind="ExternalInput")
    d_nrmT = nc.dram_tensor("nrmT", [1, NPC], f32, kind="ExternalInput")
    d_D = nc.dram_tensor("D", [NF, H], f32, kind="ExternalInput")
    d_baseT = nc.dram_tensor("baseT", [P, 2], f32, kind="ExternalInput")
    d_W = nc.dram_tensor("W", [P, L * 2 * H], bf16, kind="ExternalInput")
    d_gT = nc.dram_tensor("gT", [P, 2 * L], f32, kind="ExternalInput")
    d_bT = nc.dram_tensor("bT", [P, 2 * L], f32, kind="ExternalInput")
    d_psel = nc.dram_tensor("psel", [P, BPC * G], bf16, kind="ExternalInput")
    d_w1 = nc.dram_tensor("w1", [P, 2 * P], f32, kind="ExternalInput")
    d_w2 = nc.dram_tensor("w2", [P, 64], f32, kind="ExternalInput")
    d_w3 = nc.dram_tensor("w3", [64, 1], f32, kind="ExternalInput")
    d_b1 = nc.dram_tensor("b1", [P, 1], f32, kind="ExternalInput")
    d_b2 = nc.dram_tensor("b2", [64, 1], f32, kind="ExternalInput")
    d_b3 = nc.dram_tensor("b3", [1, 1], f32, kind="ExternalInput")
    d_out = nc.dram_tensor("out", [1, G], f32, kind="ExternalOutput")

    rg = [list(range(NCORE))]

    with tile.TileContext(nc) as tc, ExitStack() as ctx:
        pers = ctx.enter_context(tc.tile_pool(name="pers", bufs=1))
        psT = ctx.enter_context(tc.tile_pool(name="psT", bufs=1, space="PSUM"))
        psG = ctx.enter_context(tc.tile_pool(name="psG", bufs=2, space="PSUM"))
        stream = ctx.enter_context(tc.tile_pool(name="stream", bufs=3))
        work = ctx.enter_context(tc.tile_pool(name="work", bufs=2))
        dram = ctx.enter_context(tc.tile_pool(name="dram", bufs=1, space="DRAM"))

        # ---- persistent SBUF state -------------------------------------
        hT = pers.tile([P, 2 * NPC], f32, tag="hT")
        hsT = pers.tile([P, 2 * NPC], bf16, tag="hsT")
        tT = pers.tile([P, 2 * NPC], f32, tag="tT")
        nrm_rep = pers.tile([P, NPC], f32, tag="nrm_rep")
        A_sb = pers.tile([P, SB * NPC], fp8, tag="A_sb")
        xT_sb = pers.tile([NF, NPC], f32, tag="xT")
        D_sb = pers.tile([NF, H], f32, tag="D")
        baseT_sb = pers.tile([P, 2], f32, tag="baseT")
        nrmT_sb = pers.tile([1, NPC], f32, tag="nrmT")
        W_sb = pers.tile([P, L * 2 * H], bf16, tag="W")
        gT_sb = pers.tile([P, 2 * L], f32, tag="gT")
        bT_sb = pers.tile([P, 2 * L], f32, tag="bT")
        stats = pers.tile([P, 12], f32, tag="stats")
        ac_sb = pers.tile([P, 4], f32, tag="ac")
        bnw = pers.tile([P, 8], f32, tag="bnw")
        hN = pers.tile([P, BPC * H], bf16, tag="hN")
        ident_bf = pers.tile([P, P], bf16, tag="ident")
        ones128b = pers.tile([P, 1], bf16, tag="ones128b")
        ones1 = pers.tile([1, P], f32, tag="ones1")
        w1_sb = pers.tile([P, 2 * P], f32, tag="w1")
        w2_sb = pers.tile([P, 64], f32, tag="w2")
        w3_sb = pers.tile([64, 1], f32, tag="w3")
        b1_sb = pers.tile([P, 1], f32, tag="b1")
        b2_sb = pers.tile([64, 1], f32, tag="b2")
        b3_sb = pers.tile([1, 1], f32, tag="b3")
        scal = pers.tile([1, 2 * G], f32, tag="scal")

        # ---- DRAM bounce buffers ---------------------------------------
        HPC = NPC // 2  # 640 rows per half-shard
        ag_in = dram.tile([NPC, H], bf16, tag="ag_in")
        ag_outs = [
            [dram.tile([NCORE * HPC, H], bf16, tag=f"ag_out{hh}_{ll}",
                       name=f"ag_out{hh}_{ll}", addr_space="Shared")
             for hh in range(2)]
            for ll in range(L)]
        ar_in = dram.tile([P, 4], f32, tag="ar_in")
        ar_out = dram.tile([P, 4], f32, tag="ar_out")
        pr_in = dram.tile([2 * P + 1, G], f32, tag="pr_in")
        pr_out = dram.tile([2 * P + 1, G], f32, tag="pr_out")
        dummy_in = dram.tile([1, 1], f32, tag="dummy_in")
        dummy_out = dram.tile([1, 1], f32, tag="dummy_out")

        # ---- input loads ------------------------------------------------
        for t, d in [(xT_sb, d_xT), (D_sb, d_D), (baseT_sb, d_baseT),
                     (nrmT_sb, d_nrmT), (W_sb, d_W), (gT_sb, d_gT),
                     (bT_sb, d_bT), (w1_sb, d_w1), (w2_sb, d_w2),
                     (w3_sb, d_w3), (b1_sb, d_b1), (b2_sb, d_b2),
                     (b3_sb, d_b3)]:
            nc.sync.dma_start(out=t[:], in_=d[:])

        ACH = SB // 16  # 5 src blocks per A-load chunk
        for ch in range(16):
            nc.sync.dma_start(
                out=A_sb[:, ch * ACH * NPC:(ch + 1) * ACH * NPC],
                in_=d_A[:, ch * ACH * NPC:(ch + 1) * ACH * NPC])

        make_identity(nc, ident_bf[:])
        nc.vector.memset(ones128b[:], 1.0)
        nc.vector.memset(ones1[:], 1.0)

        # dummy collective: pays the one-time mesh-init cost while the
        # encoder and input loads run
        nc.sync.dma_start(out=dummy_in[:], in_=d_b3[:])
        nc.gpsimd.collective_compute(
            "AllReduce", OP.add, replica_groups=rg,
            ins=[dummy_in[:]], outs=[dummy_out[:]])

        # nrm_rep[p, j] = nrm[j] : rank-1 broadcast via f32 matmuls
        for j0 in range(0, NPC, 512):
            j1 = min(j0 + 512, NPC)
            ps_n = psT.tile([P, 512], f32, tag="sp00", name="ps_n")
            nc.tensor.matmul(out=ps_n[:, :j1 - j0], lhsT=ones1[:],
                             rhs=nrmT_sb[:, j0:j1], start=True, stop=True)
            nc.vector.tensor_copy(out=nrm_rep[:, j0:j1], in_=ps_n[:, :j1 - j0])

        # ---- encoder: hT = baseT + D^T @ xT (per h-half) ----------------
        for half in range(2):
            for p, (j0, j1) in enumerate(PASSES):
                ps_e = psT.tile([P, j1 - j0], f32, tag=f"sp{half}{p}",
                                name="ps_e")
                pe_sl = ps_e[:]
                nc.tensor.matmul(
                    out=pe_sl,
                    lhsT=D_sb[:, half * P:(half + 1) * P],
                    rhs=xT_sb[:, j0:j1], start=True, stop=True)
                hslc = hT[:, half * NPC + j0:half * NPC + j1]
                nc.vector.tensor_scalar_add(hslc, pe_sl,
                                            baseT_sb[:, half:half + 1])
                nc.vector.tensor_tensor(
                    out=hsT[:, half * NPC + j0:half * NPC + j1],
                    in0=hslc, in1=nrm_rep[:, j0:j1], op=OP.mult)

        if variant == "enc":
            nc.sync.dma_start(out=d_out[:], in_=hT[0:1, 0:G])

        # ---- layers -----------------------------------------------------
        nlayers = 0 if variant == "enc" else (1 if variant in ("ag", "l1") else L)

        def emit_gemm_ag(lw):
            """GEMM hws = hsT.T @ W[lw] -> ag_in; split AllGather into
            ag_outs[lw]. AG0 is issued as soon as its five blocks exist."""
            for nb in range(BPC):
                ps_g = psG.tile([P, H], f32, tag="mm", name="ps_g")
                for k in range(2):
                    nc.tensor.matmul(
                        out=ps_g[:],
                        lhsT=hsT[:, k * NPC + nb * P:k * NPC + (nb + 1) * P],
                        rhs=W_sb[:, (lw * 2 + k) * H:(lw * 2 + k + 1) * H],
                        start=(k == 0), stop=(k == 1))
                hws_bf = work.tile([P, H], bf16, tag="hws", name="hws_bf")
                nc.vector.tensor_copy(out=hws_bf[:], in_=ps_g[:])
                nc.sync.dma_start(out=ag_in[nb * P:(nb + 1) * P, :],
                                  in_=hws_bf[:])
                if nb == BPC // 2 - 1:
                    nc.gpsimd.collective_compute(
                        "AllGather", OP.bypass, replica_groups=rg,
                        ins=[ag_in[0:HPC, :]], outs=[ag_outs[lw][0][:]])
            nc.gpsimd.collective_compute(
                "AllGather", OP.bypass, replica_groups=rg,
                ins=[ag_in[HPC:NPC, :]], outs=[ag_outs[lw][1][:]])

        # table chunk plan: tiny leading chunks so the first SpMM matmul
        # is not gated on a large DMA
        chunk_plan = []           # (half, core, first_block, nblocks)
        chunk_of = {}             # slab pos s -> (chunk idx, local block)
        for hh in range(2):
            for k in range(NCORE):
                sizes = [1, 1, 1, 2] if (hh == 0 and k == 0) else [5]
                b0 = 0
                for sz in sizes:
                    ci = len(chunk_plan)
                    chunk_plan.append((hh, k, b0, sz))
                    for bb in range(sz):
                        chunk_of[hh * 40 + k * 5 + b0 + bb] = (ci, bb)
                    b0 += sz

        if nlayers > 0:
            emit_gemm_ag(0)
        if variant == "ag":
            sbt = work.tile([1, G], bf16, tag="dbg")
            nc.sync.dma_start(out=sbt[:], in_=ag_outs[0][0][0:1, 0:G])
            sbt2 = work.tile([1, G], f32, tag="dbg2")
            nc.vector.tensor_copy(out=sbt2[:], in_=sbt[:])
            nc.sync.dma_start(out=d_out[:], in_=sbt2[:])
            nlayers = 0

        for l in range(nlayers):
            # stream table chunks from the shared AG buffers
            tch_list = []
            for (hh, k, b0, sz) in chunk_plan:
                ago = ag_outs[l][hh]
                tch = stream.tile([P, (BPC // 2) * H], bf16, tag="tbl",
                                  bufs=8, name="tch")
                nc.sync.dma_start(
                    out=tch[:, :sz * H].rearrange("p (b h2) -> p b h2", h2=H),
                    in_=ago[k * HPC + b0 * P:k * HPC + (b0 + sz) * P, :]
                    .rearrange("(b p) h2 -> p b h2", p=P))
                tch_list.append(tch)

            # SpMM: t^T[h, d] = sum_s tbl[s, h] A[s, d]; s-major with all
            # six (half x pass) PSUM accumulators open
            ps_a = {}
            for half in range(2):
                for p, (j0, j1) in enumerate(PASSES):
                    ps_a[(half, p)] = psT.tile(
                        [P, j1 - j0], f32, tag=f"sp{half}{p}", name="ps_acc")[:]
            for s in range(SB):
                ci, bb = chunk_of[s]
                tch = tch_list[ci]
                off = bb * H
                for half in range(2):
                    lhsT = tch[:, off + half * P:off + (half + 1) * P]
                    for p, (j0, j1) in enumerate(PASSES):
                        nc.tensor.matmul(
                            out=ps_a[(half, p)],
                            lhsT=lhsT,
                            rhs=A_sb[:, s * NPC + j0:s * NPC + j1],
                            start=(s == 0), stop=(s == SB - 1))
            # nrm_dst scale + row-sum stats
            for half in range(2):
                for p, (j0, j1) in enumerate(PASSES):
                    tslc = tT[:, half * NPC + j0:half * NPC + j1]
                    nc.vector.tensor_tensor(
                        out=tslc, in0=ps_a[(half, p)],
                        in1=nrm_rep[:, j0:j1], op=OP.mult)
                    col = 2 * p + half
                    nc.vector.tensor_reduce(
                        out=stats[:, col:col + 1], in_=tslc,
                        axis=mybir.AxisListType.X, op=OP.add)
                    sq = work.tile([P, j1 - j0], f32, tag="sq")
                    nc.vector.affine_mul_reduce(
                        out=sq[:], accum_out=stats[:, 6 + col:7 + col],
                        in0=tslc, in1=tslc, scale=1.0, bias=0.0)

            # combine pass partials -> [P,4] (sum0,sum1,q0,q1), AllReduce
            pack = work.tile([P, 4], f32, tag="pack")
            nc.vector.tensor_tensor(out=pack[:, 0:2], in0=stats[:, 0:2],
                                    in1=stats[:, 2:4], op=OP.add)
            nc.vector.tensor_tensor(out=pack[:, 0:2], in0=pack[:, 0:2],
                                    in1=stats[:, 4:6], op=OP.add)
            nc.vector.tensor_tensor(out=pack[:, 2:4], in0=stats[:, 6:8],
                                    in1=stats[:, 8:10], op=OP.add)
            nc.vector.tensor_tensor(out=pack[:, 2:4], in0=pack[:, 2:4],
                                    in1=stats[:, 10:12], op=OP.add)
            nc.sync.dma_start(out=ar_in[:], in_=pack[:])
            nc.gpsimd.collective_compute(
                "AllReduce", OP.add, replica_groups=rg,
                ins=[ar_in[:]], outs=[ar_out[:]])
            stv = work.tile([P, 4], f32, tag="stv")
            nc.sync.dma_start(out=stv[:], in_=ar_out[:])

            # per-partition BN: a = gamma*istd, c = beta - mu*a
            mu = bnw[:, 0:2]
            var = bnw[:, 2:4]
            msq = bnw[:, 4:6]
            nc.vector.tensor_scalar_mul(mu, stv[:, 0:2], 1.0 / N)
            nc.vector.tensor_scalar_mul(var, stv[:, 2:4], 1.0 / N)
            nc.vector.tensor_tensor(out=msq, in0=mu, in1=mu, op=OP.mult)
            nc.vector.tensor_tensor(out=var, in0=var, in1=msq, op=OP.subtract)
            nc.vector.tensor_scalar_add(var, var, BN_EPS)
            nc.vector.reciprocal(out=var, in_=var)
            nc.scalar.activation(out=var, in_=var, func=FT.Sqrt)  # istd
            nc.vector.tensor_tensor(out=ac_sb[:, 0:2], in0=var,
                                    in1=gT_sb[:, 2 * l:2 * l + 2], op=OP.mult)
            nc.vector.tensor_tensor(out=msq, in0=mu, in1=ac_sb[:, 0:2], op=OP.mult)
            nc.vector.tensor_tensor(out=ac_sb[:, 2:4],
                                    in0=bT_sb[:, 2 * l:2 * l + 2],
                                    in1=msq, op=OP.subtract)

            # apply: h += relu(a*t + c); hs = h*nrm; next GEMM interleaved
            # per 512-column chunk so the next AllGather is issued early
            last = (l == nlayers - 1) and variant != "l1"
            nb_done = 0
            for p, (j0, j1) in enumerate(PASSES):
                for half in range(2):
                    hslc = hT[:, half * NPC + j0:half * NPC + j1]
                    tslc = tT[:, half * NPC + j0:half * NPC + j1]
                    u = work.tile([P, 512], f32, tag="u", name="u")
                    nc.scalar.activation(out=u[:, :j1 - j0], in_=tslc,
                                         func=FT.Relu,
                                         scale=ac_sb[:, half:half + 1],
                                         bias=ac_sb[:, 2 + half:3 + half])
                    nc.vector.tensor_tensor(out=hslc, in0=hslc,
                                            in1=u[:, :j1 - j0], op=OP.add)
                    if not last:
                        nc.vector.tensor_tensor(
                            out=hsT[:, half * NPC + j0:half * NPC + j1],
                            in0=hslc, in1=nrm_rep[:, j0:j1], op=OP.mult)
            if not last and l + 1 < nlayers:
                emit_gemm_ag(l + 1)

        if variant == "l1":
            nc.sync.dma_start(out=d_out[:], in_=hT[0:1, 0:G])
        skip_pool = variant in ("enc", "ag", "l1")

        # ---- pooling + MLP ---------------------------------------------
        if not skip_pool:
            # transpose all of hT back to node-major first (keeps the pool
            # matmul accumulation groups free of interleaved transposes)
            for nb in range(BPC):
                hb = work.tile([P, H], bf16, tag="hb")
                for half in range(2):
                    nc.vector.tensor_copy(
                        out=hb[:, half * P:(half + 1) * P],
                        in_=hT[:, half * NPC + nb * P:half * NPC + (nb + 1) * P])
                for half in range(2):
                    pst = psG.tile([P, P], bf16, tag="mm")
                    nc.tensor.transpose(out=pst[:], in_=hb[:, half * P:(half + 1) * P],
                                        identity=ident_bf[:])
                    nc.vector.tensor_copy(
                        out=hN[:, nb * H + half * P:nb * H + (half + 1) * P],
                        in_=pst[:])
            ps_p0 = psT.tile([P, G], f32, tag="sp00")
            ps_p1 = psT.tile([P, G], f32, tag="sp10")
            ps_pc = psT.tile([1, G], f32, tag="sp01")
            for nb in range(BPC):
                psel_t = stream.tile([P, G], bf16, tag="psel")
                nc.sync.dma_start(out=psel_t[:], in_=d_psel[:, nb * G:(nb + 1) * G])
                nc.tensor.matmul(out=ps_p0[:], lhsT=hN[:, nb * H:nb * H + P],
                                 rhs=psel_t[:],
                                 start=(nb == 0), stop=(nb == BPC - 1))
                nc.tensor.matmul(out=ps_p1[:], lhsT=hN[:, nb * H + P:(nb + 1) * H],
                                 rhs=psel_t[:],
                                 start=(nb == 0), stop=(nb == BPC - 1))
                nc.tensor.matmul(out=ps_pc[:], lhsT=ones128b[:], rhs=psel_t[:],
                                 start=(nb == 0), stop=(nb == BPC - 1))
            g0 = work.tile([P, G], f32, tag="g0")
            g1 = work.tile([P, G], f32, tag="g1")
            cnt = scal[:, 0:G]
            nc.vector.tensor_copy(out=g0[:], in_=ps_p0[:])
            nc.vector.tensor_copy(out=g1[:], in_=ps_p1[:])
            nc.vector.tensor_copy(out=cnt, in_=ps_pc[:])
            nc.sync.dma_start(out=pr_in[0:P, :], in_=g0[:])
            nc.sync.dma_start(out=pr_in[P:2 * P, :], in_=g1[:])
            nc.sync.dma_start(out=pr_in[2 * P:2 * P + 1, :], in_=cnt)
            nc.gpsimd.collective_compute(
                "AllReduce", OP.add, replica_groups=rg,
                ins=[pr_in[:]], outs=[pr_out[:]])
            nc.sync.dma_start(out=g0[:], in_=pr_out[0:P, :])
            nc.sync.dma_start(out=g1[:], in_=pr_out[P:2 * P, :])
            nc.sync.dma_start(out=cnt, in_=pr_out[2 * P:2 * P + 1, :])
            nc.vector.tensor_scalar_max(cnt, cnt, 1.0)
            nc.vector.reciprocal(out=cnt, in_=cnt)
            ps_r = psT.tile([P, G], f32, tag="sp11")
            nc.tensor.matmul(out=ps_r[:], lhsT=ones1[:], rhs=cnt, start=True, stop=True)
            rc_rep = work.tile([P, G], f32, tag="rc_rep")
            nc.vector.tensor_copy(out=rc_rep[:], in_=ps_r[:])
            nc.vector.tensor_tensor(out=g0[:], in0=g0[:], in1=rc_rep[:], op=OP.mult)
            nc.vector.tensor_tensor(out=g1[:], in0=g1[:], in1=rc_rep[:], op=OP.mult)

            # MLP head (weights as lhsT, graphs along free dim)
            ps1 = psT.tile([P, G], f32, tag="sp02")
            nc.tensor.matmul(out=ps1[:], lhsT=w1_sb[:, 0:P], rhs=g0[:], start=True, stop=False)
            nc.tensor.matmul(out=ps1[:], lhsT=w1_sb[:, P:2 * P], rhs=g1[:], start=False, stop=True)
            y1 = work.tile([P, G], f32, tag="y1")
            nc.scalar.activation(out=y1[:], in_=ps1[:], func=FT.Relu, bias=b1_sb[:, 0:1])
            ps2 = psT.tile([64, G], f32, tag="sp12")
            nc.tensor.matmul(out=ps2[:], lhsT=w2_sb[:], rhs=y1[:], start=True, stop=True)
            y2 = work.tile([64, G], f32, tag="y2")
            nc.scalar.activation(out=y2[:], in_=ps2[:], func=FT.Relu, bias=b2_sb[:, 0:1])
            ps3 = psT.tile([1, G], f32, tag="sp01")
            nc.tensor.matmul(out=ps3[:], lhsT=w3_sb[:], rhs=y2[:], start=True, stop=True)
            y3 = work.tile([1, G], f32, tag="y3")
            nc.vector.tensor_scalar_add(y3[:], ps3[:], b3_sb[0:1, 0:1])
            nc.sync.dma_start(out=d_out[:], in_=y3[:])

    nc.compile()
    return nc


# --------------------------------------------------------------------------
# entry point
# --------------------------------------------------------------------------

def kernel(x, edge_index, batch_ids, emb, W, b, gamma, beta,
           mlp_W1, mlp_b1, mlp_W2, mlp_b2, mlp_W3, mlp_b3,
           _trace=False, _trace_kwargs=None):
    # NB: reference BN subtracts the per-channel mean, so the additive bias b
    # cancels exactly and is not needed by the device program.
    in_maps = _preprocess(x, edge_index, batch_ids, emb, W, gamma, beta,
                          mlp_W1, mlp_b1, mlp_W2, mlp_b2, mlp_W3, mlp_b3)
    import os
    variant = os.environ.get("KVARIANT", "full")
    if variant not in _compiled:
        _compiled[variant] = _build(variant)
    nc = _compiled[variant]
    kw = {}
    if _trace:
        kw = dict(trace=True, **(_trace_kwargs or {}))
    res = run_bass_kernel_spmd(nc, in_maps, core_ids=list(range(NCORE)), **kw)
    out = np.asarray(res.results[0]["out"], np.float32).reshape(G, 1)
    kernel._last_results = res
    return out
